# revision 25
# baseline (speedup 1.0000x reference)
"""Trainium2 Bass kernel for nn_DiffusionActionHead (B=8, S=2048, D=4096).

v3 strategy (8 NeuronCores, batch-parallel + head-parallel):
  - Host folds weight-only math:  U = wk^T (probe@wq + bq) / sqrt(DH)
    (removes wq/wk and the U AllGather);  w2rin = mlp_w2 @ rin_w[pool]
    ((attn_out+h) is consumed only through rin_w -> the 16 MiB w2 stream
    becomes 1 MiB and the mlp AllReduce becomes the 8 KiB x0 AllReduce);
    LN affine gains fold into w1 / blk_w1 rows (y_aff@W = y_core@(g*W) +
    (b@W folded into the bias)).
  - Scores stream llm^T in fp8 e3m4 (softmax washes the quantization to
    ~0.2% on attention weights); pooled streams llm natural in f16.
  - All m=8 matmuls 4-way column-tiled (tile_position, measured 2.35x).
  - Pooled runs in two D-halves with two pipelined AllToAlls; ctx
    consumes each half as it lands.  x0 partials (attn_part@rin_pool,
    computed during the attn AllReduce) collapse into one 8 KiB AR.
  - Biases enter PSUM via 128-row replicated bias tile (ones/128) so all
    matmuls keep the (128,32) PE tiling mode.
  - Rings: scalar = llm streams + wo + odd w1; sync = rin_pool + even w1
    + w2rin + tail weights; gpsimd = smalls, wv, collective bounces
    (with f16<->f32 casts on the attn AllReduce wire).
"""

import numpy as np
import sys

if "/opt/trn_rl_repo" not in sys.path:
    sys.path.insert(0, "/opt/trn_rl_repo")

import ml_dtypes
import concourse.bass as bass
import concourse.tile as tile
from concourse import bacc, mybir
from concourse.masks import make_identity
from concourse.bass_utils import run_bass_kernel_spmd

F32 = mybir.dt.float32
F16 = mybir.dt.float16
F8 = mybir.dt.float8e3
NP8 = ml_dtypes.float8_e3m4
AF = mybir.ActivationFunctionType
ALU = mybir.AluOpType

B, S, D = 8, 2048, 4096
H, AD, TD, HID, NBLK = 8, 7, 32, 256, 3
DH = D // H
NC = 8
P = 128
SC = S // P            # 16
DC = D // P            # 32
HD2 = D // 2           # 2048
F1S = 4 * D // NC      # 2048
FC = F1S // P          # 16
HC = HID // P          # 2
SU = 2048.0
RSQRT_DH = 1.0 / float(np.sqrt(DH))
TWO_PI = 2.0 * float(np.pi)

BO_O = 0
B1_O = 4096
BV_O = 6144
BB1_O = 6656
BB2_O = 9728
BREP_N = 10496


def _bcast(src_ap, nparts):
    ap = src_ap
    assert ap.shape[0] == 1, ap.shape
    return bass.AP(tensor=ap.tensor, offset=ap.offset,
                   ap=[[0, nparts]] + [list(x) for x in ap.ap[1:]])


def build_program():
    nc = bacc.Bacc("TRN2", target_bir_lowering=False, debug=False,
                   num_devices=NC)
    t = {}

    def din(name, shape, dtype=F32):
        t[name] = nc.dram_tensor(name, shape, dtype, kind="ExternalInput")

    din("llm16", [S, D], F16)
    din("llmT8", [D, S], F8)
    din("U8r", [P, DC, H], F8)
    din("wv16", [8, P, 4, DH], F16)
    din("wo16", [DH, D], F16)
    din("w116", [16, P, 2, F1S], F16)
    din("w2rin16", [P, FC, HID], F16)
    din("rp16", [P, DC, HID], F16)
    din("brep", [1, BREP_N], F16)
    din("four_w2", [TD, 1]); din("phase2", [TD, 1])
    din("timeT", [1, B]); din("naT", [AD, B], F16)
    din("cw1", [TD, 2 * TD], F16); din("cb1c", [2 * TD, 1])
    din("cw2rin8", [2 * TD, HID], F16)
    din("rna8", [AD, HID], F16)
    din("rb8", [1, HID], F16)
    din("bw1a", [P, 3, 4 * HID], F16)
    din("bw1b", [P, 3, 4 * HID], F16)
    din("bw2a", [P, 12, HID], F16)
    din("bw2b", [P, 12, HID], F16)
    din("ow", [P, HC, AD], F16); din("out_bc", [1, AD])
    t["res"] = nc.dram_tensor("res", [B, AD], F32, kind="ExternalOutput")

    t["cc_pool_in"] = nc.dram_tensor("cc_pool_in", [H, D], F16)
    t["cc_pool_out"] = nc.dram_tensor("cc_pool_out", [B, D], F16)
    t["cc_attn_in"] = nc.dram_tensor("cc_attn_in", [B, D], F32)
    t["cc_attn_out"] = nc.dram_tensor("cc_attn_out", [B, D], F32,
                                      addr_space="Shared")
    t["cc_x0_in"] = nc.dram_tensor("cc_x0_in", [B, HID], F32)
    t["cc_x0_out"] = nc.dram_tensor("cc_x0_out", [B, HID], F32,
                                    addr_space="Shared")

    with tile.TileContext(nc) as tc:
        import contextlib
        with contextlib.ExitStack() as ctx:
            _build(nc, tc, t, ctx)
    nc.finalize()
    return nc


def _build(nc, tc, t, ctx):
    GROUPS = [list(range(NC))]

    singles = ctx.enter_context(tc.tile_pool(name="singles", bufs=1))
    lt8p = ctx.enter_context(tc.tile_pool(name="lt8p", bufs=2))
    ln16pA = ctx.enter_context(tc.tile_pool(name="ln16pA", bufs=2))
    ln16pB = ctx.enter_context(tc.tile_pool(name="ln16pB", bufs=1))
    natp = ctx.enter_context(tc.tile_pool(name="natp", bufs=2))
    wvp = ctx.enter_context(tc.tile_pool(name="wvp", bufs=8))
    wop = ctx.enter_context(tc.tile_pool(name="wop", bufs=2))
    w1p = ctx.enter_context(tc.tile_pool(name="w1p", bufs=4))
    psA = ctx.enter_context(tc.tile_pool(name="psA", bufs=2, space="PSUM"))
    psB = ctx.enter_context(tc.tile_pool(name="psB", bufs=2, space="PSUM"))
    psT8 = ctx.enter_context(tc.tile_pool(name="psT8", bufs=2, space="PSUM"))

    ident = singles.tile([P, P], F32)
    make_identity(nc, ident)
    ident16 = singles.tile([P, P], F16)
    nc.vector.tensor_copy(out=ident16[:], in_=ident[:])
    eps_sb = singles.tile([P, 1], F32)
    nc.vector.memset(eps_sb[:], 1e-5)
    ones8 = singles.tile([1, 8], F16)
    nc.vector.memset(ones8[:], 1.0)
    ones128 = singles.tile([P, 8], F16)
    nc.vector.memset(ones128[:], 1.0 / 128.0)

    def t_nat_to_T(src_nat, dst_T, nchunks, npart, uid, evict_eng=None,
                   c0=0):
        eng = evict_eng or nc.vector
        for c in range(nchunks):
            ps = psT8.tile([P, 16], F16, tag="tp16", name=f"tp_{uid}_{c}")
            nc.tensor.transpose(ps[:, :npart], src_nat[:, c * P:(c + 1) * P],
                                ident16[:npart, :npart])
            if eng is nc.scalar:
                nc.scalar.activation(out=dst_T[:, c0 + c, :],
                                     in_=ps[:, :npart], func=AF.Identity)
            else:
                eng.tensor_copy(out=dst_T[:, c0 + c, :], in_=ps[:, :npart])

    def bias_rep(ps_slice, col0, n_total, tp, stop):
        nchn = (n_total + 511) // 512
        for n in range(nchn):
            w = min(512, n_total - n * 512)
            nc.tensor.matmul(
                ps_slice[:, n * 512:n * 512 + w], ones128[:, :B],
                brep_sb[:, col0 + n * 512:col0 + n * 512 + w],
                start=False, stop=stop, tile_position=tp)

    def quad_sum(dst, ps, n):
        nc.vector.tensor_copy(out=dst, in_=ps[0:B, :n])
        nc.vector.tensor_add(out=dst, in0=dst, in1=ps[32:32 + B, :n])
        nc.vector.tensor_add(out=dst, in0=dst, in1=ps[64:64 + B, :n])
        nc.vector.tensor_add(out=dst, in0=dst, in1=ps[96:96 + B, :n])

    def layernorm_nat(x_nat, npart, n, y_nat, uid):
        nsub = max(1, n // 512)
        st = singles.tile([npart, nsub, nc.vector.BN_STATS_DIM], F32,
                          name=f"lnst_{uid}")
        xg = x_nat.rearrange("p (a b) -> p a b", a=nsub)
        for g in range(nsub):
            nc.vector.bn_stats(out=st[:, g, :], in_=xg[:, g, :])
        mv = singles.tile([npart, nc.vector.BN_AGGR_DIM], F32,
                          name=f"lnmv_{uid}")
        nc.vector.bn_aggr(out=mv[:], in_=st[:])
        std = singles.tile([npart, 1], F32, name=f"lnsd_{uid}")
        nc.scalar.activation(out=std[:], in_=mv[:, 1:2], func=AF.Sqrt,
                             bias=eps_sb[:npart, :])
        nc.vector.reciprocal(out=std[:], in_=std[:])
        nc.vector.tensor_scalar(out=y_nat, in0=x_nat, scalar1=mv[:, 0:1],
                                scalar2=std[:], op0=ALU.subtract, op1=ALU.mult)

    # ===== STEP 0: U + rp on sync; llmT8 stream hoisted on scalar; smalls
    # on gpsimd (cond inputs first, bulky brep last).
    u8_sb = singles.tile([P, DC, H], F8)
    nc.sync.dma_start(out=u8_sb[:], in_=t["U8r"][:])
    rp_sb = singles.tile([P, DC, HID], F16)
    nc.sync.dma_start(out=rp_sb[:], in_=t["rp16"][:])

    lt_tiles = []
    for g in range(8):
        lt = lt8p.tile([P, 4, S], F8, tag="lt8", name=f"lt8_{g}")
        nc.scalar.dma_start(
            out=lt[:],
            in_=t["llmT8"][g * 512:(g + 1) * 512, :].rearrange(
                "(c p) s -> p c s", p=P))
        lt_tiles.append(lt)

    fw_sb = singles.tile([TD, 1], F32)
    nc.gpsimd.dma_start(out=fw_sb[:], in_=t["four_w2"][:])
    ph_sb = singles.tile([TD, 1], F32)
    nc.gpsimd.dma_start(out=ph_sb[:], in_=t["phase2"][:])
    tb32 = singles.tile([TD, B], F32)
    nc.gpsimd.dma_start(out=tb32[:], in_=_bcast(t["timeT"][:], TD))
    cw1_sb = singles.tile([TD, 2 * TD], F16)
    nc.gpsimd.dma_start(out=cw1_sb[:], in_=t["cw1"][:])
    cb1_sb = singles.tile([2 * TD, 1], F32)
    nc.gpsimd.dma_start(out=cb1_sb[:], in_=t["cb1c"][:])
    cwr_sb = singles.tile([2 * TD, HID], F16)
    nc.gpsimd.dma_start(out=cwr_sb[:], in_=t["cw2rin8"][:])
    naT_sb = singles.tile([AD, B], F16)
    nc.gpsimd.dma_start(out=naT_sb[:], in_=t["naT"][:])
    rna_sb = singles.tile([AD, HID], F16)
    nc.gpsimd.dma_start(out=rna_sb[:], in_=t["rna8"][:])
    rb_sb = singles.tile([1, HID], F16)
    nc.gpsimd.dma_start(out=rb_sb[:], in_=t["rb8"][:])
    wv_tiles = []
    for g in range(8):
        wt = wvp.tile([P, 4, DH], F16, tag="wv", name=f"wv_{g}")
        nc.gpsimd.dma_start(out=wt[:], in_=t["wv16"][g])
        wv_tiles.append(wt)
    brep_sb = singles.tile([P, BREP_N], F16)
    nc.gpsimd.dma_start(out=brep_sb[:], in_=_bcast(t["brep"][:], P))
    ow_sb = singles.tile([P, HC, AD], F16)
    nc.gpsimd.dma_start(out=ow_sb[:], in_=t["ow"][:])
    ob_bc = singles.tile([B, AD], F32)
    nc.gpsimd.dma_start(out=ob_bc[:], in_=_bcast(t["out_bc"][:], B))

    # ===== STEP 1: scoresT = (U*SU)^T @ llmT  [fp8, tiles-over-n]
    ps_sc = psA.tile([P, 1024], F32, tag="psA", name="ps_sc")
    for g in range(8):
        for cc in range(4):
            c = 4 * g + cc
            for j in range(4):
                nc.tensor.matmul(
                    ps_sc[32 * j:32 * j + H, 0:512],
                    u8_sb[:, c, :], lt_tiles[g][:, cc, 512 * j:512 * (j + 1)],
                    start=(c == 0), stop=(c == DC - 1),
                    tile_position=(0, 32 * j))

    p_nat = natp.tile([H, S], F16, tag="nat8", name="p_nat")
    for j in range(4):
        nc.scalar.activation(out=p_nat[:, 512 * j:512 * (j + 1)],
                             in_=ps_sc[32 * j:32 * j + H, 0:512], func=AF.Exp,
                             scale=1.0 / SU)
    den = singles.tile([H, 1], F32)
    nc.vector.reduce_sum(out=den[:], in_=p_nat[:], axis=mybir.AxisListType.X)
    nc.vector.reciprocal(out=den[:], in_=den[:])
    pT = singles.tile([P, SC, H], F16)
    t_nat_to_T(p_nat, pT, SC, H, "p")

    # ===== STEP 2: pooled = pT^T @ llm / den.  Full-row 1 MiB tiles,
    # even chunks stream on sync, odd on scalar; both column-halves
    # accumulate concurrently in two psum banks; one 64 KiB AllToAll.
    ps_pool = [psA.tile([P, 1024], F32, tag="psA", name=f"ps_pool{hf}")
               for hf in range(2)]
    for c in range(SC):
        pool = ln16pA if c % 2 == 0 else ln16pB
        eng = nc.sync if c % 2 == 0 else nc.scalar
        lt = pool.tile([P, D], F16, tag="ln16", name=f"ln16_{c}")
        eng.dma_start(out=lt[:], in_=t["llm16"][c * P:(c + 1) * P, :])
        for hf in range(2):
            for j in range(4):
                n0 = hf * HD2 + 512 * j
                nc.tensor.matmul(
                    ps_pool[hf][32 * j:32 * j + H, 0:512],
                    pT[:, c, :], lt[:, n0:n0 + 512],
                    start=(c == 0), stop=(c == SC - 1),
                    tile_position=(0, 32 * j))
    pooled = natp.tile([H, D], F16, tag="natD", name="pooled")
    for hf in range(2):
        for j in range(4):
            nc.vector.tensor_scalar(
                out=pooled[:, hf * HD2 + 512 * j:hf * HD2 + 512 * (j + 1)],
                in0=ps_pool[hf][32 * j:32 * j + H, 0:512],
                scalar1=den[:], scalar2=None, op0=ALU.mult)
    nc.gpsimd.dma_start(out=t["cc_pool_in"][:], in_=pooled[:])
    nc.gpsimd.collective_compute(
        "AllToAll", ALU.bypass, replica_groups=GROUPS,
        ins=[t["cc_pool_in"][:].opt()], outs=[t["cc_pool_out"][:].opt()])
    poolh0 = natp.tile([B, D], F16, tag="natD", name="poolh0")
    nc.gpsimd.dma_start(out=poolh0[:], in_=t["cc_pool_out"][:])
    poolh = [poolh0]

    # ---- cond path (off critical path; PE slots in while streams run)
    fu = singles.tile([TD, B], F32)
    nc.vector.tensor_scalar_mul(out=fu[:], in0=tb32[:], scalar1=fw_sb[:])
    fi = singles.tile([TD, B], mybir.dt.int32)
    nc.vector.tensor_copy(out=fi[:], in_=fu[:])
    fif = singles.tile([TD, B], F32)
    nc.vector.tensor_copy(out=fif[:], in_=fi[:])
    nc.vector.tensor_sub(out=fu[:], in0=fu[:], in1=fif[:])
    ffT = singles.tile([TD, B], F16)
    nc.scalar.activation(out=ffT[:], in_=fu[:], func=AF.Sin,
                         scale=TWO_PI, bias=ph_sb[:])
    ps_c1 = psB.tile([P, 512], F32, tag="psB", name="ps_c1")
    nc.tensor.matmul(ps_c1[:2 * TD, :B], cw1_sb[:], ffT[:], start=True,
                     stop=True)
    c1 = singles.tile([2 * TD, B], F16)
    nc.scalar.activation(out=c1[:], in_=ps_c1[:2 * TD, :B], func=AF.Silu,
                         bias=cb1_sb[:])
    ps_e = psB.tile([P, 512], F32, tag="psB", name="ps_e")
    nc.tensor.matmul(ps_e[:B, :HID], c1[:], cwr_sb[:], start=True, stop=False)
    nc.tensor.matmul(ps_e[:B, :HID], naT_sb[:], rna_sb[:], start=False,
                     stop=False)
    nc.tensor.matmul(ps_e[:B, :HID], ones8[:, :B], rb_sb[:], start=False,
                     stop=True)
    x0_early = singles.tile([B, HID], F32)
    nc.vector.tensor_copy(out=x0_early[:], in_=ps_e[:B, :HID])

    # ===== STEP 3: ctx = poolh @ wv + bv  [tiles-over-k]
    poolhT = singles.tile([P, DC, B], F16)
    t_nat_to_T(poolh[0], poolhT, DC, B, "ph")
    ps_cx = psB.tile([P, 512], F32, tag="psB", name="ps_cx")
    for g in range(8):
        for cc in range(4):
            c = 4 * g + cc
            j = c % 4
            nc.tensor.matmul(ps_cx[32 * j:32 * j + B, :],
                             poolhT[:, c, :], wv_tiles[g][:, cc, :],
                             start=(c < 4),
                             stop=(c >= DC - 4 and j != 0),
                             tile_position=(0, 32 * j))
    bias_rep(ps_cx[0:B, :], BV_O, DH, (0, 0), stop=True)
    ctx_nat = natp.tile([B, DH], F16, tag="nat8", name="ctx_nat")
    quad_sum(ctx_nat[:], ps_cx, DH)
    ctxT = singles.tile([P, DH // P, B], F16)
    t_nat_to_T(ctx_nat, ctxT, DH // P, B, "cx")

    # ===== STEP 4: attn partial = ctx @ wo + bo/8 ; AllReduce (f32 wire)
    wo_tiles = []
    for c in range(4):
        wt = wop.tile([P, D], F16, tag="wo", name=f"wo_{c}")
        nc.gpsimd.dma_start(out=wt[:], in_=t["wo16"][c * P:(c + 1) * P, :])
        wo_tiles.append(wt)
    ps_at = psA.tile([P, 1024], F32, tag="psA", name="ps_at")
    for c in range(4):
        for j in range(4):
            for u in range(2):
                n0 = 1024 * j + 512 * u
                nc.tensor.matmul(
                    ps_at[32 * j:32 * j + B, 512 * u:512 * (u + 1)],
                    ctxT[:, c, :], wo_tiles[c][:, n0:n0 + 512],
                    start=(c == 0), stop=False,
                    tile_position=(0, 32 * j))
    for j in range(4):
        bias_rep(ps_at[32 * j:32 * j + B, :], BO_O + 1024 * j, 1024,
                 (0, 32 * j), stop=True)
    attn_part = natp.tile([B, D], F16, tag="natD", name="attn_part")
    for j in range(4):
        nc.scalar.activation(out=attn_part[:, 1024 * j:1024 * (j + 1)],
                             in_=ps_at[32 * j:32 * j + B, :],
                             func=AF.Identity)
    # w1 chunks 0..15 stream on sync (window fills from T~10); chunks
    # 16..31 ride the freed wv pool slots on gpsimd during the AllReduce.
    w1_tiles = []
    for g in range(8):
        wt = w1p.tile([P, 2, F1S], F16, tag="w1", name=f"w1_{g}")
        nc.sync.dma_start(out=wt[:], in_=t["w116"][g])
        w1_tiles.append(wt)
    nc.gpsimd.dma_start(out=t["cc_attn_in"][:], in_=attn_part[:])
    nc.gpsimd.collective_compute(
        "AllReduce", ALU.add, replica_groups=GROUPS,
        ins=[t["cc_attn_in"][:].opt()], outs=[t["cc_attn_out"][:].opt()])
    attn_nat = natp.tile([B, D], F16, tag="natD", name="attn_nat")
    nc.gpsimd.dma_start(out=attn_nat[:], in_=t["cc_attn_out"][:])
    w1g_tiles = []
    for k in range(16):
        wt = wvp.tile([P, F1S], F16, tag="wv", name=f"w1g_{k}")
        nc.gpsimd.dma_start(out=wt[:], in_=t["w116"][8 + k // 2, :, k % 2, :])
        w1g_tiles.append(wt)

    # overlapped with the AllReduce: x0 += attn_partial @ rin_pool
    apT = singles.tile([P, DC, B], F16)
    t_nat_to_T(attn_part, apT, DC, B, "ap")
    ps_xa = psB.tile([P, 512], F32, tag="psB", name="ps_xa")
    for c in range(DC):
        j = c % 4
        nc.tensor.matmul(ps_xa[32 * j:32 * j + B, :HID], apT[:, c, :],
                         rp_sb[:, c, :], start=(c < 4), stop=(c >= DC - 4),
                         tile_position=(0, 32 * j))
    xa_nat = singles.tile([B, HID], F32)
    quad_sum(xa_nat[:], ps_xa, HID)
    nc.vector.tensor_add(out=x0_early[:], in0=x0_early[:], in1=xa_nat[:])

    # ===== STEP 5: y = LN(attn) (affine folded into w1) ; mm1
    y_nat = natp.tile([B, D], F16, tag="natD", name="y_nat")
    layernorm_nat(attn_nat[:], B, D, y_nat[:], "ln0")
    yT = singles.tile([P, DC, B], F16)
    t_nat_to_T(y_nat, yT, DC, B, "y")

    ps_h1 = psA.tile([P, 1024], F32, tag="psA", name="ps_h1")
    for c in range(DC):
        src = (w1_tiles[c // 2][:, c % 2, :] if c < 16
               else w1g_tiles[c - 16][:, :])
        for j in range(4):
            nc.tensor.matmul(
                ps_h1[32 * j:32 * j + B, 0:512],
                yT[:, c, :], src[:, 512 * j:512 * (j + 1)],
                start=(c == 0), stop=False,
                tile_position=(0, 32 * j))
    for j in range(4):
        bias_rep(ps_h1[32 * j:32 * j + B, 0:512], B1_O + 512 * j, 512,
                 (0, 32 * j), stop=True)
    g_nat = natp.tile([B, F1S], F16, tag="nat8", name="g_nat")
    for j in range(4):
        nc.scalar.activation(out=g_nat[:, 512 * j:512 * (j + 1)],
                             in_=ps_h1[32 * j:32 * j + B, 0:512],
                             func=AF.Gelu)
    gT = singles.tile([P, FC, B], F16)
    t_nat_to_T(g_nat, gT, FC, B, "g")

    # ===== STEP 6: x0 += g1 @ w2rin ; AllReduce(x0)
    w2r_sb = w1p.tile([P, FC, HID], F16, tag="w2r", bufs=1, name="w2r")
    nc.sync.dma_start(out=w2r_sb[:], in_=t["w2rin16"][:])
    ps_x0 = psB.tile([P, 512], F32, tag="psB", name="ps_x0")
    for c in range(FC):
        j = c % 4
        nc.tensor.matmul(ps_x0[32 * j:32 * j + B, :HID], gT[:, c, :],
                         w2r_sb[:, c, :], start=(c < 4), stop=(c >= FC - 4),
                         tile_position=(0, 32 * j))
    xg_nat = singles.tile([B, HID], F32)
    quad_sum(xg_nat[:], ps_x0, HID)
    nc.vector.tensor_add(out=x0_early[:], in0=x0_early[:], in1=xg_nat[:])
    nc.gpsimd.dma_start(out=t["cc_x0_in"][:], in_=x0_early[:])
    nc.gpsimd.collective_compute(
        "AllReduce", ALU.add, replica_groups=GROUPS,
        ins=[t["cc_x0_in"][:].opt()], outs=[t["cc_x0_out"][:].opt()])

    # ===== STEP 7: diffusion tail (replicated; blk LN affine folded)
    bw1a = w1p.tile([P, 3, 4 * HID], F16, tag="w1", name="bw1a")
    nc.sync.dma_start(out=bw1a[:], in_=t["bw1a"][:])
    bw1b = w1p.tile([P, 3, 4 * HID], F16, tag="w1", name="bw1b")
    nc.sync.dma_start(out=bw1b[:], in_=t["bw1b"][:])
    bw2a = w1p.tile([P, 12, HID], F16, tag="w1", name="bw2a")
    nc.sync.dma_start(out=bw2a[:], in_=t["bw2a"][:])
    bw2b = w1p.tile([P, 12, HID], F16, tag="w1", name="bw2b")
    nc.sync.dma_start(out=bw2b[:], in_=t["bw2b"][:])

    x_nat = singles.tile([B, HID], F32)
    nc.gpsimd.dma_start(out=x_nat[:], in_=t["cc_x0_out"][:])

    for i in range(NBLK):
        xn = singles.tile([B, HID], F16, name=f"xn_{i}")
        layernorm_nat(x_nat[:], B, HID, xn[:], f"lnb{i}")
        xnT = singles.tile([P, HC, B], F16, name=f"xnT_{i}")
        t_nat_to_T(xn, xnT, HC, B, f"xn{i}")
        ps_bh = psB.tile([P, 512], F32, tag="psB", name=f"ps_bh_{i}")
        for j in range(4):
            for c in range(HC):
                f = 2 * i + c
                src = bw1a if f < 3 else bw1b
                nc.tensor.matmul(
                    ps_bh[32 * j:32 * j + B, 0:256],
                    xnT[:, c, :], src[:, f % 3, 256 * j:256 * (j + 1)],
                    start=(c == 0), stop=False,
                    tile_position=(0, 32 * j))
            bias_rep(ps_bh[32 * j:32 * j + B, 0:256],
                     BB1_O + 1024 * i + 256 * j, 256, (0, 32 * j), stop=True)
        hb = natp.tile([B, 4 * HID], F16, tag="nat8", name=f"hb_{i}")
        for j in range(4):
            nc.scalar.activation(out=hb[:, 256 * j:256 * (j + 1)],
                                 in_=ps_bh[32 * j:32 * j + B, 0:256],
                                 func=AF.Silu)
        hbT = singles.tile([P, 4 * HID // P, B], F16, name=f"hbT_{i}")
        t_nat_to_T(hb, hbT, 4 * HID // P, B, f"hb{i}")

        ps_bo = psB.tile([P, 512], F32, tag="psB", name=f"ps_bo_{i}")
        for c in range(4 * HID // P):
            j = c % 4
            f = 8 * i + c
            src = bw2a if f < 12 else bw2b
            nc.tensor.matmul(ps_bo[32 * j:32 * j + B, :HID], hbT[:, c, :],
                             src[:, f % 12, :],
                             start=(c < 4), stop=(c >= 4 and j != 0),
                             tile_position=(0, 32 * j))
        bias_rep(ps_bo[0:B, :HID], BB2_O + 256 * i, HID, (0, 0), stop=True)
        xr = singles.tile([B, HID], F32, name=f"xr_{i}")
        quad_sum(xr[:], ps_bo, HID)
        nc.vector.tensor_add(out=x_nat[:], in0=x_nat[:], in1=xr[:])

    xs = singles.tile([B, HID], F16)
    nc.scalar.activation(out=xs[:], in_=x_nat[:], func=AF.Silu)
    xsT = singles.tile([P, HC, B], F16)
    t_nat_to_T(xs, xsT, HC, B, "xs")
    ps_o = psB.tile([P, 512], F32, tag="psB", name="ps_o")
    for c in range(HC):
        nc.tensor.matmul(ps_o[:B, :AD], xsT[:, c, :], ow_sb[:, c, :],
                         start=(c == 0), stop=(c == HC - 1))
    out_sb = singles.tile([B, AD], F32)
    nc.vector.tensor_add(out=out_sb[:], in0=ps_o[:B, :AD], in1=ob_bc[:])
    nc.sync.dma_start(out=t["res"][:], in_=out_sb[:])


_CACHED_NC = None


def _get_nc():
    global _CACHED_NC
    if _CACHED_NC is None:
        _CACHED_NC = build_program()
    return _CACHED_NC


def _prep_in_maps(inputs):
    f32 = np.float32
    f16 = np.float16
    llm_full = np.asarray(inputs["llm_output"], dtype=f32)
    wq = np.asarray(inputs["wq"], f32); wk = np.asarray(inputs["wk"], f32)
    wv = np.asarray(inputs["wv"], f32); wo = np.asarray(inputs["wo"], f32)
    bq = np.asarray(inputs["bq"], f32); bv = np.asarray(inputs["bv"], f32)
    bo = np.asarray(inputs["bo"], f32)
    ln_g = np.asarray(inputs["ln_g"], f32)
    ln_b = np.asarray(inputs["ln_b"], f32)
    w1 = np.asarray(inputs["mlp_w1"], f32); b1 = np.asarray(inputs["mlp_b1"], f32)
    w2 = np.asarray(inputs["mlp_w2"], f32); b2 = np.asarray(inputs["mlp_b2"], f32)
    rin_w = np.asarray(inputs["rin_w"], f32)
    rin_b = np.asarray(inputs["rin_b"], f32)
    probe = np.asarray(inputs["probe"], f32).reshape(D)
    cw2 = np.asarray(inputs["cond_w2"], f32)
    cb2 = np.asarray(inputs["cond_b2"], f32)
    blk_g = np.asarray(inputs["blk_ln_g"], f32)
    blk_b = np.asarray(inputs["blk_ln_b"], f32)
    blk_w1 = np.asarray(inputs["blk_w1"], f32)
    blk_w2 = np.asarray(inputs["blk_w2"], f32)
    blk_b1 = np.asarray(inputs["blk_b1"], f32)
    blk_b2 = np.asarray(inputs["blk_b2"], f32)

    # ---- weight-only folds ----
    q = (probe @ wq + bq) * RSQRT_DH
    U = np.zeros((D, H), f32)
    for h in range(H):
        U[:, h] = wk[:, h * DH:(h + 1) * DH] @ q[h * DH:(h + 1) * DH]
    U8 = (U * SU).astype(NP8)
    rin_cond = rin_w[0:TD]
    rin_pool = np.ascontiguousarray(rin_w[TD:TD + D])
    rin_na = rin_w[TD + D:]
    w2rin = w2 @ rin_pool
    cw2rin = cw2 @ rin_cond
    rb_fold = (rin_b + b2 @ rin_pool + cb2 @ rin_cond) / NC
    # LN affine folds: y_aff @ W = y_core @ (g*W) + b@W
    w1_aff = ln_g[:, None] * w1              # (D, 4D)
    b1_aff = b1 + ln_b @ w1                  # (4D,)
    bw1_aff = blk_g[:, :, None] * blk_w1     # (3, HID, 4HID)
    bb1_aff = blk_b1 + np.einsum('ih,ihf->if', blk_b, blk_w1)

    def ptile(m, c_per_g):
        K, N = m.shape
        G = K // (P * c_per_g)
        r = np.ascontiguousarray(
            m.reshape(G, c_per_g, P, N).transpose(0, 2, 1, 3))
        return r if G > 1 else r[0]

    shared = {
        "rp16": np.ascontiguousarray(
            rin_pool.reshape(DC, P, HID).transpose(1, 0, 2)).astype(f16),
        "four_w2": np.concatenate(
            [np.asarray(inputs["four_w"], f32).reshape(TD // 2, 1)] * 2),
        "phase2": np.concatenate(
            [np.full((TD // 2, 1), np.pi / 2, f32),
             np.zeros((TD // 2, 1), f32)]),
        "timeT": np.ascontiguousarray(np.asarray(inputs["time"], f32).T),
        "naT": np.ascontiguousarray(
            np.asarray(inputs["noisy_actions"], f32).T).astype(f16),
        "cw1": np.asarray(inputs["cond_w1"], f32).astype(f16),
        "cb1c": np.asarray(inputs["cond_b1"], f32).reshape(-1, 1),
        "cw2rin8": (cw2rin / NC).astype(f16),
        "rna8": (rin_na / NC).astype(f16),
        "rb8": rb_fold.astype(f16).reshape(1, HID),
        "bw1a": np.ascontiguousarray(
            bw1_aff.reshape(NBLK * HC, P, 4 * HID)[0:3].transpose(1, 0, 2)
        ).astype(f16),
        "bw1b": np.ascontiguousarray(
            bw1_aff.reshape(NBLK * HC, P, 4 * HID)[3:6].transpose(1, 0, 2)
        ).astype(f16),
        "bw2a": np.ascontiguousarray(
            blk_w2.reshape(NBLK * 8, P, HID)[0:12].transpose(1, 0, 2)
        ).astype(f16),
        "bw2b": np.ascontiguousarray(
            blk_w2.reshape(NBLK * 8, P, HID)[12:24].transpose(1, 0, 2)
        ).astype(f16),
        "ow": np.ascontiguousarray(
            np.asarray(inputs["out_w"], f32).reshape(HC, P, AD)
            .transpose(1, 0, 2)).astype(f16),
        "out_bc": np.asarray(inputs["out_b"], f32).reshape(1, AD),
        "U8r": np.ascontiguousarray(U8.reshape(DC, P, H).transpose(1, 0, 2)),
    }

    in_maps = []
    for i in range(NC):
        hb_ = slice(i * DH, (i + 1) * DH)
        fb = slice(i * F1S, (i + 1) * F1S)
        m = dict(shared)
        m["llm16"] = llm_full[i].astype(f16)
        m["llmT8"] = np.ascontiguousarray(llm_full[i].T).astype(NP8)
        m["wv16"] = ptile(np.ascontiguousarray(wv[:, hb_]), 4).astype(f16)
        m["wo16"] = np.ascontiguousarray(wo[hb_, :]).astype(f16)
        m["w116"] = ptile(np.ascontiguousarray(w1_aff[:, fb]), 2).astype(f16)
        m["w2rin16"] = ptile(np.ascontiguousarray(w2rin[fb]), FC).astype(f16)
        brep = np.zeros((1, BREP_N), f16)
        brep[0, BO_O:BO_O + D] = (bo / NC).astype(f16)
        brep[0, B1_O:B1_O + F1S] = b1_aff[fb].astype(f16)
        brep[0, BV_O:BV_O + DH] = bv[hb_].astype(f16)
        brep[0, BB1_O:BB1_O + NBLK * 4 * HID] = bb1_aff.reshape(-1).astype(f16)
        brep[0, BB2_O:BB2_O + NBLK * HID] = blk_b2.reshape(-1).astype(f16)
        m["brep"] = brep
        in_maps.append(m)
    return in_maps


def kernel(**inputs):
    nc = _get_nc()
    in_maps = _prep_in_maps(inputs)
    r = run_bass_kernel_spmd(nc, in_maps, core_ids=list(range(NC)))
    return np.ascontiguousarray(r.results[0]["res"]).astype(np.float32)


def run_traced(**inputs):
    nc = _get_nc()
    in_maps = _prep_in_maps(inputs)
    r = run_bass_kernel_spmd(nc, in_maps, core_ids=list(range(NC)), trace=True)
    return np.ascontiguousarray(r.results[0]["res"]).astype(np.float32), r


# revision 26
# speedup vs baseline: 1.0249x; 1.0249x over previous
"""Trainium2 Bass kernel for nn_DiffusionActionHead (B=8, S=2048, D=4096).

v3 strategy (8 NeuronCores, batch-parallel + head-parallel):
  - Host folds weight-only math:  U = wk^T (probe@wq + bq) / sqrt(DH)
    (removes wq/wk and the U AllGather);  w2rin = mlp_w2 @ rin_w[pool]
    ((attn_out+h) is consumed only through rin_w -> the 16 MiB w2 stream
    becomes 1 MiB and the mlp AllReduce becomes the 8 KiB x0 AllReduce);
    LN affine gains fold into w1 / blk_w1 rows (y_aff@W = y_core@(g*W) +
    (b@W folded into the bias)).
  - Scores stream llm^T in fp8 e3m4 (softmax washes the quantization to
    ~0.2% on attention weights); pooled streams llm natural in f16.
  - All m=8 matmuls 4-way column-tiled (tile_position, measured 2.35x).
  - Pooled runs in two D-halves with two pipelined AllToAlls; ctx
    consumes each half as it lands.  x0 partials (attn_part@rin_pool,
    computed during the attn AllReduce) collapse into one 8 KiB AR.
  - Biases enter PSUM via 128-row replicated bias tile (ones/128) so all
    matmuls keep the (128,32) PE tiling mode.
  - Rings: scalar = llm streams + wo + odd w1; sync = rin_pool + even w1
    + w2rin + tail weights; gpsimd = smalls, wv, collective bounces
    (with f16<->f32 casts on the attn AllReduce wire).
"""

import numpy as np
import sys

if "/opt/trn_rl_repo" not in sys.path:
    sys.path.insert(0, "/opt/trn_rl_repo")

import ml_dtypes
import concourse.bass as bass
import concourse.tile as tile
from concourse import bacc, mybir
from concourse.masks import make_identity
from concourse.bass_utils import run_bass_kernel_spmd

F32 = mybir.dt.float32
F16 = mybir.dt.float16
F8 = mybir.dt.float8e3
NP8 = ml_dtypes.float8_e3m4
AF = mybir.ActivationFunctionType
ALU = mybir.AluOpType

B, S, D = 8, 2048, 4096
H, AD, TD, HID, NBLK = 8, 7, 32, 256, 3
DH = D // H
NC = 8
P = 128
SC = S // P            # 16
DC = D // P            # 32
HD2 = D // 2           # 2048
F1S = 4 * D // NC      # 2048
FC = F1S // P          # 16
HC = HID // P          # 2
SU = 2048.0
RSQRT_DH = 1.0 / float(np.sqrt(DH))
TWO_PI = 2.0 * float(np.pi)

BO_O = 0
B1_O = 4096
BV_O = 6144
BB1_O = 6656
BB2_O = 9728
BREP_N = 10496


def _bcast(src_ap, nparts):
    ap = src_ap
    assert ap.shape[0] == 1, ap.shape
    return bass.AP(tensor=ap.tensor, offset=ap.offset,
                   ap=[[0, nparts]] + [list(x) for x in ap.ap[1:]])


def build_program():
    nc = bacc.Bacc("TRN2", target_bir_lowering=False, debug=False,
                   num_devices=NC)
    t = {}

    def din(name, shape, dtype=F32):
        t[name] = nc.dram_tensor(name, shape, dtype, kind="ExternalInput")

    din("llm16", [S, D], F16)
    din("llmT8", [D, S], F8)
    din("U8r", [P, DC, H], F8)
    din("wv16", [8, P, 4, DH], F16)
    din("wo16", [DH, D], F16)
    din("w116", [16, P, 2, F1S], F16)
    din("w2rin16", [P, FC, HID], F16)
    din("rp16", [P, DC, HID], F16)
    din("brep", [1, BREP_N], F16)
    din("four_w2", [TD, 1]); din("phase2", [TD, 1])
    din("timeT", [1, B]); din("naT", [AD, B], F16)
    din("cw1", [TD, 2 * TD], F16); din("cb1c", [2 * TD, 1])
    din("cw2rin8", [2 * TD, HID], F16)
    din("rna8", [AD, HID], F16)
    din("rb8", [1, HID], F16)
    din("bw1a", [P, 3, 4 * HID], F16)
    din("bw1b", [P, 3, 4 * HID], F16)
    din("bw2a", [P, 12, HID], F16)
    din("bw2b", [P, 12, HID], F16)
    din("ow", [P, HC, AD], F16); din("out_bc", [1, AD])
    t["res"] = nc.dram_tensor("res", [B, AD], F32, kind="ExternalOutput")

    t["cc_pool_in"] = nc.dram_tensor("cc_pool_in", [H, D], F16)
    t["cc_pool_out"] = nc.dram_tensor("cc_pool_out", [B, D], F16)
    t["cc_attn_in"] = nc.dram_tensor("cc_attn_in", [B, D], F32)
    t["cc_attn_out"] = nc.dram_tensor("cc_attn_out", [B, D], F32,
                                      addr_space="Shared")
    t["cc_x0_in"] = nc.dram_tensor("cc_x0_in", [B, HID], F32)
    t["cc_x0_out"] = nc.dram_tensor("cc_x0_out", [B, HID], F32,
                                    addr_space="Shared")

    with tile.TileContext(nc) as tc:
        import contextlib
        with contextlib.ExitStack() as ctx:
            _build(nc, tc, t, ctx)
    nc.finalize()
    return nc


def _build(nc, tc, t, ctx):
    GROUPS = [list(range(NC))]

    singles = ctx.enter_context(tc.tile_pool(name="singles", bufs=1))
    lt8p = ctx.enter_context(tc.tile_pool(name="lt8p", bufs=2))
    ln16pA = ctx.enter_context(tc.tile_pool(name="ln16pA", bufs=3))
    ln16pB = ctx.enter_context(tc.tile_pool(name="ln16pB", bufs=3))
    natp = ctx.enter_context(tc.tile_pool(name="natp", bufs=2))
    wvp = ctx.enter_context(tc.tile_pool(name="wvp", bufs=8))
    w1p = ctx.enter_context(tc.tile_pool(name="w1p", bufs=3))
    psA = ctx.enter_context(tc.tile_pool(name="psA", bufs=2, space="PSUM"))
    psB = ctx.enter_context(tc.tile_pool(name="psB", bufs=2, space="PSUM"))
    psT8 = ctx.enter_context(tc.tile_pool(name="psT8", bufs=2, space="PSUM"))

    ident = singles.tile([P, P], F32)
    make_identity(nc, ident)
    ident16 = singles.tile([P, P], F16)
    nc.vector.tensor_copy(out=ident16[:], in_=ident[:])
    eps_sb = singles.tile([P, 1], F32)
    nc.vector.memset(eps_sb[:], 1e-5)
    ones8 = singles.tile([1, 8], F16)
    nc.vector.memset(ones8[:], 1.0)
    ones128 = singles.tile([P, 8], F16)
    nc.vector.memset(ones128[:], 1.0 / 128.0)

    def t_nat_to_T(src_nat, dst_T, nchunks, npart, uid, evict_eng=None,
                   c0=0):
        eng = evict_eng or nc.vector
        for c in range(nchunks):
            ps = psT8.tile([P, 16], F16, tag="tp16", name=f"tp_{uid}_{c}")
            nc.tensor.transpose(ps[:, :npart], src_nat[:, c * P:(c + 1) * P],
                                ident16[:npart, :npart])
            if eng is nc.scalar:
                nc.scalar.activation(out=dst_T[:, c0 + c, :],
                                     in_=ps[:, :npart], func=AF.Identity)
            else:
                eng.tensor_copy(out=dst_T[:, c0 + c, :], in_=ps[:, :npart])

    def bias_rep(ps_slice, col0, n_total, tp, stop):
        nchn = (n_total + 511) // 512
        for n in range(nchn):
            w = min(512, n_total - n * 512)
            nc.tensor.matmul(
                ps_slice[:, n * 512:n * 512 + w], ones128[:, :B],
                brep_sb[:, col0 + n * 512:col0 + n * 512 + w],
                start=False, stop=stop, tile_position=tp)

    def quad_sum(dst, ps, n):
        nc.vector.tensor_copy(out=dst, in_=ps[0:B, :n])
        nc.vector.tensor_add(out=dst, in0=dst, in1=ps[32:32 + B, :n])
        nc.vector.tensor_add(out=dst, in0=dst, in1=ps[64:64 + B, :n])
        nc.vector.tensor_add(out=dst, in0=dst, in1=ps[96:96 + B, :n])

    def layernorm_nat(x_nat, npart, n, y_nat, uid):
        nsub = max(1, n // 512)
        st = singles.tile([npart, nsub, nc.vector.BN_STATS_DIM], F32,
                          name=f"lnst_{uid}")
        xg = x_nat.rearrange("p (a b) -> p a b", a=nsub)
        for g in range(nsub):
            nc.vector.bn_stats(out=st[:, g, :], in_=xg[:, g, :])
        mv = singles.tile([npart, nc.vector.BN_AGGR_DIM], F32,
                          name=f"lnmv_{uid}")
        nc.vector.bn_aggr(out=mv[:], in_=st[:])
        std = singles.tile([npart, 1], F32, name=f"lnsd_{uid}")
        nc.scalar.activation(out=std[:], in_=mv[:, 1:2], func=AF.Sqrt,
                             bias=eps_sb[:npart, :])
        nc.vector.reciprocal(out=std[:], in_=std[:])
        nc.vector.tensor_scalar(out=y_nat, in0=x_nat, scalar1=mv[:, 0:1],
                                scalar2=std[:], op0=ALU.subtract, op1=ALU.mult)

    # ===== STEP 0: U + rp on sync; llmT8 stream hoisted on scalar; smalls
    # on gpsimd (cond inputs first, bulky brep last).
    u8_sb = singles.tile([P, DC, H], F8)
    nc.sync.dma_start(out=u8_sb[:], in_=t["U8r"][:])
    rp_sb = singles.tile([P, DC, HID], F16)
    nc.sync.dma_start(out=rp_sb[:], in_=t["rp16"][:])

    lt_tiles = []
    for g in range(8):
        lt = lt8p.tile([P, 4, S], F8, tag="lt8", name=f"lt8_{g}")
        nc.scalar.dma_start(
            out=lt[:],
            in_=t["llmT8"][g * 512:(g + 1) * 512, :].rearrange(
                "(c p) s -> p c s", p=P))
        lt_tiles.append(lt)

    fw_sb = singles.tile([TD, 1], F32)
    nc.gpsimd.dma_start(out=fw_sb[:], in_=t["four_w2"][:])
    ph_sb = singles.tile([TD, 1], F32)
    nc.gpsimd.dma_start(out=ph_sb[:], in_=t["phase2"][:])
    tb32 = singles.tile([TD, B], F32)
    nc.gpsimd.dma_start(out=tb32[:], in_=_bcast(t["timeT"][:], TD))
    cw1_sb = singles.tile([TD, 2 * TD], F16)
    nc.gpsimd.dma_start(out=cw1_sb[:], in_=t["cw1"][:])
    cb1_sb = singles.tile([2 * TD, 1], F32)
    nc.gpsimd.dma_start(out=cb1_sb[:], in_=t["cb1c"][:])
    cwr_sb = singles.tile([2 * TD, HID], F16)
    nc.gpsimd.dma_start(out=cwr_sb[:], in_=t["cw2rin8"][:])
    naT_sb = singles.tile([AD, B], F16)
    nc.gpsimd.dma_start(out=naT_sb[:], in_=t["naT"][:])
    rna_sb = singles.tile([AD, HID], F16)
    nc.gpsimd.dma_start(out=rna_sb[:], in_=t["rna8"][:])
    rb_sb = singles.tile([1, HID], F16)
    nc.gpsimd.dma_start(out=rb_sb[:], in_=t["rb8"][:])
    wv_tiles = []
    for g in range(8):
        wt = wvp.tile([P, 4, DH], F16, tag="wv", name=f"wv_{g}")
        nc.gpsimd.dma_start(out=wt[:], in_=t["wv16"][g])
        wv_tiles.append(wt)
    brep_sb = singles.tile([P, BREP_N], F16)
    nc.gpsimd.dma_start(out=brep_sb[:], in_=_bcast(t["brep"][:], P))
    ow_sb = singles.tile([P, HC, AD], F16)
    nc.gpsimd.dma_start(out=ow_sb[:], in_=t["ow"][:])
    ob_bc = singles.tile([B, AD], F32)
    nc.gpsimd.dma_start(out=ob_bc[:], in_=_bcast(t["out_bc"][:], B))

    # ===== STEP 1: scoresT = (U*SU)^T @ llmT  [fp8, tiles-over-n]
    ps_sc = psA.tile([P, 1024], F32, tag="psA", name="ps_sc")
    for g in range(8):
        for cc in range(4):
            c = 4 * g + cc
            for j in range(4):
                nc.tensor.matmul(
                    ps_sc[32 * j:32 * j + H, 0:512],
                    u8_sb[:, c, :], lt_tiles[g][:, cc, 512 * j:512 * (j + 1)],
                    start=(c == 0), stop=(c == DC - 1),
                    tile_position=(0, 32 * j))

    p_nat = natp.tile([H, S], F16, tag="nat8", name="p_nat")
    for j in range(4):
        nc.scalar.activation(out=p_nat[:, 512 * j:512 * (j + 1)],
                             in_=ps_sc[32 * j:32 * j + H, 0:512], func=AF.Exp,
                             scale=1.0 / SU)
    den = singles.tile([H, 1], F32)
    nc.vector.reduce_sum(out=den[:], in_=p_nat[:], axis=mybir.AxisListType.X)
    nc.vector.reciprocal(out=den[:], in_=den[:])
    pT = singles.tile([P, SC, H], F16)
    t_nat_to_T(p_nat, pT, SC, H, "p")

    # ===== STEP 2: pooled = pT^T @ llm / den.  Full-row 1 MiB tiles,
    # even chunks stream on sync, odd on scalar; both column-halves
    # accumulate concurrently in two psum banks; one 64 KiB AllToAll.
    ps_pool = [psA.tile([P, 1024], F32, tag="psA", name=f"ps_pool{hf}")
               for hf in range(2)]
    for c in range(SC):
        pool = ln16pA if c % 2 == 0 else ln16pB
        eng = nc.sync if c % 2 == 0 else nc.scalar
        lt = pool.tile([P, D], F16, tag="ln16", name=f"ln16_{c}")
        eng.dma_start(out=lt[:], in_=t["llm16"][c * P:(c + 1) * P, :])
        for hf in range(2):
            for j in range(4):
                n0 = hf * HD2 + 512 * j
                nc.tensor.matmul(
                    ps_pool[hf][32 * j:32 * j + H, 0:512],
                    pT[:, c, :], lt[:, n0:n0 + 512],
                    start=(c == 0), stop=(c == SC - 1),
                    tile_position=(0, 32 * j))
    pooled = natp.tile([H, D], F16, tag="natD", name="pooled")
    for hf in range(2):
        for j in range(4):
            nc.vector.tensor_scalar(
                out=pooled[:, hf * HD2 + 512 * j:hf * HD2 + 512 * (j + 1)],
                in0=ps_pool[hf][32 * j:32 * j + H, 0:512],
                scalar1=den[:], scalar2=None, op0=ALU.mult)
    nc.gpsimd.dma_start(out=t["cc_pool_in"][:], in_=pooled[:])
    nc.gpsimd.collective_compute(
        "AllToAll", ALU.bypass, replica_groups=GROUPS,
        ins=[t["cc_pool_in"][:].opt()], outs=[t["cc_pool_out"][:].opt()])
    poolh0 = natp.tile([B, D], F16, tag="natD", name="poolh0")
    nc.gpsimd.dma_start(out=poolh0[:], in_=t["cc_pool_out"][:])
    poolh = [poolh0]

    # ---- cond path (off critical path; PE slots in while streams run)
    fu = singles.tile([TD, B], F32)
    nc.vector.tensor_scalar_mul(out=fu[:], in0=tb32[:], scalar1=fw_sb[:])
    fi = singles.tile([TD, B], mybir.dt.int32)
    nc.vector.tensor_copy(out=fi[:], in_=fu[:])
    fif = singles.tile([TD, B], F32)
    nc.vector.tensor_copy(out=fif[:], in_=fi[:])
    nc.vector.tensor_sub(out=fu[:], in0=fu[:], in1=fif[:])
    ffT = singles.tile([TD, B], F16)
    nc.scalar.activation(out=ffT[:], in_=fu[:], func=AF.Sin,
                         scale=TWO_PI, bias=ph_sb[:])
    ps_c1 = psB.tile([P, 512], F32, tag="psB", name="ps_c1")
    nc.tensor.matmul(ps_c1[:2 * TD, :B], cw1_sb[:], ffT[:], start=True,
                     stop=True)
    c1 = singles.tile([2 * TD, B], F16)
    nc.scalar.activation(out=c1[:], in_=ps_c1[:2 * TD, :B], func=AF.Silu,
                         bias=cb1_sb[:])
    ps_e = psB.tile([P, 512], F32, tag="psB", name="ps_e")
    nc.tensor.matmul(ps_e[:B, :HID], c1[:], cwr_sb[:], start=True, stop=False)
    nc.tensor.matmul(ps_e[:B, :HID], naT_sb[:], rna_sb[:], start=False,
                     stop=False)
    nc.tensor.matmul(ps_e[:B, :HID], ones8[:, :B], rb_sb[:], start=False,
                     stop=True)
    x0_early = singles.tile([B, HID], F32)
    nc.vector.tensor_copy(out=x0_early[:], in_=ps_e[:B, :HID])

    # ===== STEP 3: ctx = poolh @ wv + bv  [tiles-over-k]
    poolhT = singles.tile([P, DC, B], F16)
    t_nat_to_T(poolh[0], poolhT, DC, B, "ph")
    ps_cx = psB.tile([P, 512], F32, tag="psB", name="ps_cx")
    for g in range(8):
        for cc in range(4):
            c = 4 * g + cc
            j = c % 4
            nc.tensor.matmul(ps_cx[32 * j:32 * j + B, :],
                             poolhT[:, c, :], wv_tiles[g][:, cc, :],
                             start=(c < 4),
                             stop=(c >= DC - 4 and j != 0),
                             tile_position=(0, 32 * j))
    bias_rep(ps_cx[0:B, :], BV_O, DH, (0, 0), stop=True)
    ctx_nat = natp.tile([B, DH], F16, tag="nat8", name="ctx_nat")
    quad_sum(ctx_nat[:], ps_cx, DH)
    ctxT = singles.tile([P, DH // P, B], F16)
    t_nat_to_T(ctx_nat, ctxT, DH // P, B, "cx")

    # ===== STEP 4: attn partial = ctx @ wo + bo/8 ; AllReduce (f32 wire)
    wo_tiles = []
    for c in range(4):
        for hf in range(2):
            wt = wvp.tile([P, HD2], F16, tag="wv", name=f"wo_{c}_{hf}")
            nc.gpsimd.dma_start(
                out=wt[:],
                in_=t["wo16"][c * P:(c + 1) * P, hf * HD2:(hf + 1) * HD2])
            wo_tiles.append(wt)
    ps_at = psA.tile([P, 1024], F32, tag="psA", name="ps_at")
    for c in range(4):
        for j in range(4):
            for u in range(2):
                n0 = 1024 * j + 512 * u
                nc.tensor.matmul(
                    ps_at[32 * j:32 * j + B, 512 * u:512 * (u + 1)],
                    ctxT[:, c, :],
                    wo_tiles[2 * c + n0 // HD2][:, n0 % HD2:n0 % HD2 + 512],
                    start=(c == 0), stop=False,
                    tile_position=(0, 32 * j))
    for j in range(4):
        bias_rep(ps_at[32 * j:32 * j + B, :], BO_O + 1024 * j, 1024,
                 (0, 32 * j), stop=True)
    attn_part = natp.tile([B, D], F16, tag="natD", name="attn_part")
    for j in range(4):
        nc.scalar.activation(out=attn_part[:, 1024 * j:1024 * (j + 1)],
                             in_=ps_at[32 * j:32 * j + B, :],
                             func=AF.Identity)
    # w1 chunks 0..15 stream on sync (window fills from T~10); chunks
    # 16..31 ride the freed wv pool slots on gpsimd during the AllReduce.
    w1_tiles = []
    for g in range(8):
        wt = w1p.tile([P, 2, F1S], F16, tag="w1", name=f"w1_{g}")
        nc.sync.dma_start(out=wt[:], in_=t["w116"][g])
        w1_tiles.append(wt)
    nc.gpsimd.dma_start(out=t["cc_attn_in"][:], in_=attn_part[:])
    nc.gpsimd.collective_compute(
        "AllReduce", ALU.add, replica_groups=GROUPS,
        ins=[t["cc_attn_in"][:].opt()], outs=[t["cc_attn_out"][:].opt()])
    attn_nat = natp.tile([B, D], F16, tag="natD", name="attn_nat")
    nc.gpsimd.dma_start(out=attn_nat[:], in_=t["cc_attn_out"][:])
    w1g_tiles = []
    for k in range(16):
        wt = wvp.tile([P, F1S], F16, tag="wv", name=f"w1g_{k}")
        nc.gpsimd.dma_start(out=wt[:], in_=t["w116"][8 + k // 2, :, k % 2, :])
        w1g_tiles.append(wt)

    # overlapped with the AllReduce: x0 += attn_partial @ rin_pool
    apT = singles.tile([P, DC, B], F16)
    t_nat_to_T(attn_part, apT, DC, B, "ap")
    ps_xa = psB.tile([P, 512], F32, tag="psB", name="ps_xa")
    for c in range(DC):
        j = c % 4
        nc.tensor.matmul(ps_xa[32 * j:32 * j + B, :HID], apT[:, c, :],
                         rp_sb[:, c, :], start=(c < 4), stop=(c >= DC - 4),
                         tile_position=(0, 32 * j))
    xa_nat = singles.tile([B, HID], F32)
    quad_sum(xa_nat[:], ps_xa, HID)
    nc.vector.tensor_add(out=x0_early[:], in0=x0_early[:], in1=xa_nat[:])

    # ===== STEP 5: y = LN(attn) (affine folded into w1) ; mm1
    y_nat = natp.tile([B, D], F16, tag="natD", name="y_nat")
    layernorm_nat(attn_nat[:], B, D, y_nat[:], "ln0")
    yT = singles.tile([P, DC, B], F16)
    t_nat_to_T(y_nat, yT, DC, B, "y")

    ps_h1 = psA.tile([P, 1024], F32, tag="psA", name="ps_h1")
    for c in range(DC):
        src = (w1_tiles[c // 2][:, c % 2, :] if c < 16
               else w1g_tiles[c - 16][:, :])
        for j in range(4):
            nc.tensor.matmul(
                ps_h1[32 * j:32 * j + B, 0:512],
                yT[:, c, :], src[:, 512 * j:512 * (j + 1)],
                start=(c == 0), stop=False,
                tile_position=(0, 32 * j))
    for j in range(4):
        bias_rep(ps_h1[32 * j:32 * j + B, 0:512], B1_O + 512 * j, 512,
                 (0, 32 * j), stop=True)
    g_nat = natp.tile([B, F1S], F16, tag="nat8", name="g_nat")
    for j in range(4):
        nc.scalar.activation(out=g_nat[:, 512 * j:512 * (j + 1)],
                             in_=ps_h1[32 * j:32 * j + B, 0:512],
                             func=AF.Gelu)
    gT = singles.tile([P, FC, B], F16)
    t_nat_to_T(g_nat, gT, FC, B, "g")

    # ===== STEP 6: x0 += g1 @ w2rin ; AllReduce(x0)
    w2r_sb = w1p.tile([P, FC, HID], F16, tag="w2r", bufs=1, name="w2r")
    nc.sync.dma_start(out=w2r_sb[:], in_=t["w2rin16"][:])
    ps_x0 = psB.tile([P, 512], F32, tag="psB", name="ps_x0")
    for c in range(FC):
        j = c % 4
        nc.tensor.matmul(ps_x0[32 * j:32 * j + B, :HID], gT[:, c, :],
                         w2r_sb[:, c, :], start=(c < 4), stop=(c >= FC - 4),
                         tile_position=(0, 32 * j))
    xg_nat = singles.tile([B, HID], F32)
    quad_sum(xg_nat[:], ps_x0, HID)
    nc.vector.tensor_add(out=x0_early[:], in0=x0_early[:], in1=xg_nat[:])
    nc.gpsimd.dma_start(out=t["cc_x0_in"][:], in_=x0_early[:])
    nc.gpsimd.collective_compute(
        "AllReduce", ALU.add, replica_groups=GROUPS,
        ins=[t["cc_x0_in"][:].opt()], outs=[t["cc_x0_out"][:].opt()])

    # ===== STEP 7: diffusion tail (replicated; blk LN affine folded)
    bw1a = w1p.tile([P, 3, 4 * HID], F16, tag="w1", name="bw1a")
    nc.sync.dma_start(out=bw1a[:], in_=t["bw1a"][:])
    bw1b = w1p.tile([P, 3, 4 * HID], F16, tag="w1", name="bw1b")
    nc.sync.dma_start(out=bw1b[:], in_=t["bw1b"][:])
    bw2a = w1p.tile([P, 12, HID], F16, tag="w1", name="bw2a")
    nc.sync.dma_start(out=bw2a[:], in_=t["bw2a"][:])
    bw2b = w1p.tile([P, 12, HID], F16, tag="w1", name="bw2b")
    nc.sync.dma_start(out=bw2b[:], in_=t["bw2b"][:])

    x_nat = singles.tile([B, HID], F32)
    nc.gpsimd.dma_start(out=x_nat[:], in_=t["cc_x0_out"][:])

    for i in range(NBLK):
        xn = singles.tile([B, HID], F16, name=f"xn_{i}")
        layernorm_nat(x_nat[:], B, HID, xn[:], f"lnb{i}")
        xnT = singles.tile([P, HC, B], F16, name=f"xnT_{i}")
        t_nat_to_T(xn, xnT, HC, B, f"xn{i}")
        ps_bh = psB.tile([P, 512], F32, tag="psB", name=f"ps_bh_{i}")
        for j in range(4):
            for c in range(HC):
                f = 2 * i + c
                src = bw1a if f < 3 else bw1b
                nc.tensor.matmul(
                    ps_bh[32 * j:32 * j + B, 0:256],
                    xnT[:, c, :], src[:, f % 3, 256 * j:256 * (j + 1)],
                    start=(c == 0), stop=False,
                    tile_position=(0, 32 * j))
            bias_rep(ps_bh[32 * j:32 * j + B, 0:256],
                     BB1_O + 1024 * i + 256 * j, 256, (0, 32 * j), stop=True)
        hb = natp.tile([B, 4 * HID], F16, tag="nat8", name=f"hb_{i}")
        for j in range(4):
            nc.scalar.activation(out=hb[:, 256 * j:256 * (j + 1)],
                                 in_=ps_bh[32 * j:32 * j + B, 0:256],
                                 func=AF.Silu)
        hbT = singles.tile([P, 4 * HID // P, B], F16, name=f"hbT_{i}")
        t_nat_to_T(hb, hbT, 4 * HID // P, B, f"hb{i}")

        ps_bo = psB.tile([P, 512], F32, tag="psB", name=f"ps_bo_{i}")
        for c in range(4 * HID // P):
            j = c % 4
            f = 8 * i + c
            src = bw2a if f < 12 else bw2b
            nc.tensor.matmul(ps_bo[32 * j:32 * j + B, :HID], hbT[:, c, :],
                             src[:, f % 12, :],
                             start=(c < 4), stop=(c >= 4 and j != 0),
                             tile_position=(0, 32 * j))
        bias_rep(ps_bo[0:B, :HID], BB2_O + 256 * i, HID, (0, 0), stop=True)
        for q in range(4):
            nc.vector.tensor_add(out=x_nat[:], in0=x_nat[:],
                                 in1=ps_bo[32 * q:32 * q + B, :HID])

    xs = singles.tile([B, HID], F16)
    nc.scalar.activation(out=xs[:], in_=x_nat[:], func=AF.Silu)
    xsT = singles.tile([P, HC, B], F16)
    t_nat_to_T(xs, xsT, HC, B, "xs")
    ps_o = psB.tile([P, 512], F32, tag="psB", name="ps_o")
    for c in range(HC):
        nc.tensor.matmul(ps_o[:B, :AD], xsT[:, c, :], ow_sb[:, c, :],
                         start=(c == 0), stop=(c == HC - 1))
    out_sb = singles.tile([B, AD], F32)
    nc.vector.tensor_add(out=out_sb[:], in0=ps_o[:B, :AD], in1=ob_bc[:])
    nc.sync.dma_start(out=t["res"][:], in_=out_sb[:])


_CACHED_NC = None


def _get_nc():
    global _CACHED_NC
    if _CACHED_NC is None:
        _CACHED_NC = build_program()
    return _CACHED_NC


def _prep_in_maps(inputs):
    f32 = np.float32
    f16 = np.float16
    llm_full = np.asarray(inputs["llm_output"], dtype=f32)
    wq = np.asarray(inputs["wq"], f32); wk = np.asarray(inputs["wk"], f32)
    wv = np.asarray(inputs["wv"], f32); wo = np.asarray(inputs["wo"], f32)
    bq = np.asarray(inputs["bq"], f32); bv = np.asarray(inputs["bv"], f32)
    bo = np.asarray(inputs["bo"], f32)
    ln_g = np.asarray(inputs["ln_g"], f32)
    ln_b = np.asarray(inputs["ln_b"], f32)
    w1 = np.asarray(inputs["mlp_w1"], f32); b1 = np.asarray(inputs["mlp_b1"], f32)
    w2 = np.asarray(inputs["mlp_w2"], f32); b2 = np.asarray(inputs["mlp_b2"], f32)
    rin_w = np.asarray(inputs["rin_w"], f32)
    rin_b = np.asarray(inputs["rin_b"], f32)
    probe = np.asarray(inputs["probe"], f32).reshape(D)
    cw2 = np.asarray(inputs["cond_w2"], f32)
    cb2 = np.asarray(inputs["cond_b2"], f32)
    blk_g = np.asarray(inputs["blk_ln_g"], f32)
    blk_b = np.asarray(inputs["blk_ln_b"], f32)
    blk_w1 = np.asarray(inputs["blk_w1"], f32)
    blk_w2 = np.asarray(inputs["blk_w2"], f32)
    blk_b1 = np.asarray(inputs["blk_b1"], f32)
    blk_b2 = np.asarray(inputs["blk_b2"], f32)

    # ---- weight-only folds ----
    q = (probe @ wq + bq) * RSQRT_DH
    U = np.zeros((D, H), f32)
    for h in range(H):
        U[:, h] = wk[:, h * DH:(h + 1) * DH] @ q[h * DH:(h + 1) * DH]
    U8 = (U * SU).astype(NP8)
    rin_cond = rin_w[0:TD]
    rin_pool = np.ascontiguousarray(rin_w[TD:TD + D])
    rin_na = rin_w[TD + D:]
    w2rin = w2 @ rin_pool
    cw2rin = cw2 @ rin_cond
    rb_fold = (rin_b + b2 @ rin_pool + cb2 @ rin_cond) / NC
    # LN affine folds: y_aff @ W = y_core @ (g*W) + b@W
    w1_aff = ln_g[:, None] * w1              # (D, 4D)
    b1_aff = b1 + ln_b @ w1                  # (4D,)
    bw1_aff = blk_g[:, :, None] * blk_w1     # (3, HID, 4HID)
    bb1_aff = blk_b1 + np.einsum('ih,ihf->if', blk_b, blk_w1)

    def ptile(m, c_per_g):
        K, N = m.shape
        G = K // (P * c_per_g)
        r = np.ascontiguousarray(
            m.reshape(G, c_per_g, P, N).transpose(0, 2, 1, 3))
        return r if G > 1 else r[0]

    shared = {
        "rp16": np.ascontiguousarray(
            rin_pool.reshape(DC, P, HID).transpose(1, 0, 2)).astype(f16),
        "four_w2": np.concatenate(
            [np.asarray(inputs["four_w"], f32).reshape(TD // 2, 1)] * 2),
        "phase2": np.concatenate(
            [np.full((TD // 2, 1), np.pi / 2, f32),
             np.zeros((TD // 2, 1), f32)]),
        "timeT": np.ascontiguousarray(np.asarray(inputs["time"], f32).T),
        "naT": np.ascontiguousarray(
            np.asarray(inputs["noisy_actions"], f32).T).astype(f16),
        "cw1": np.asarray(inputs["cond_w1"], f32).astype(f16),
        "cb1c": np.asarray(inputs["cond_b1"], f32).reshape(-1, 1),
        "cw2rin8": (cw2rin / NC).astype(f16),
        "rna8": (rin_na / NC).astype(f16),
        "rb8": rb_fold.astype(f16).reshape(1, HID),
        "bw1a": np.ascontiguousarray(
            bw1_aff.reshape(NBLK * HC, P, 4 * HID)[0:3].transpose(1, 0, 2)
        ).astype(f16),
        "bw1b": np.ascontiguousarray(
            bw1_aff.reshape(NBLK * HC, P, 4 * HID)[3:6].transpose(1, 0, 2)
        ).astype(f16),
        "bw2a": np.ascontiguousarray(
            blk_w2.reshape(NBLK * 8, P, HID)[0:12].transpose(1, 0, 2)
        ).astype(f16),
        "bw2b": np.ascontiguousarray(
            blk_w2.reshape(NBLK * 8, P, HID)[12:24].transpose(1, 0, 2)
        ).astype(f16),
        "ow": np.ascontiguousarray(
            np.asarray(inputs["out_w"], f32).reshape(HC, P, AD)
            .transpose(1, 0, 2)).astype(f16),
        "out_bc": np.asarray(inputs["out_b"], f32).reshape(1, AD),
        "U8r": np.ascontiguousarray(U8.reshape(DC, P, H).transpose(1, 0, 2)),
    }

    in_maps = []
    for i in range(NC):
        hb_ = slice(i * DH, (i + 1) * DH)
        fb = slice(i * F1S, (i + 1) * F1S)
        m = dict(shared)
        m["llm16"] = llm_full[i].astype(f16)
        m["llmT8"] = np.ascontiguousarray(llm_full[i].T).astype(NP8)
        m["wv16"] = ptile(np.ascontiguousarray(wv[:, hb_]), 4).astype(f16)
        m["wo16"] = np.ascontiguousarray(wo[hb_, :]).astype(f16)
        m["w116"] = ptile(np.ascontiguousarray(w1_aff[:, fb]), 2).astype(f16)
        m["w2rin16"] = ptile(np.ascontiguousarray(w2rin[fb]), FC).astype(f16)
        brep = np.zeros((1, BREP_N), f16)
        brep[0, BO_O:BO_O + D] = (bo / NC).astype(f16)
        brep[0, B1_O:B1_O + F1S] = b1_aff[fb].astype(f16)
        brep[0, BV_O:BV_O + DH] = bv[hb_].astype(f16)
        brep[0, BB1_O:BB1_O + NBLK * 4 * HID] = bb1_aff.reshape(-1).astype(f16)
        brep[0, BB2_O:BB2_O + NBLK * HID] = blk_b2.reshape(-1).astype(f16)
        m["brep"] = brep
        in_maps.append(m)
    return in_maps


def kernel(**inputs):
    nc = _get_nc()
    in_maps = _prep_in_maps(inputs)
    r = run_bass_kernel_spmd(nc, in_maps, core_ids=list(range(NC)))
    return np.ascontiguousarray(r.results[0]["res"]).astype(np.float32)


def run_traced(**inputs):
    nc = _get_nc()
    in_maps = _prep_in_maps(inputs)
    r = run_bass_kernel_spmd(nc, in_maps, core_ids=list(range(NC)), trace=True)
    return np.ascontiguousarray(r.results[0]["res"]).astype(np.float32), r


# revision 28
# speedup vs baseline: 1.1130x; 1.0859x over previous
"""Trainium2 Bass kernel for nn_DiffusionActionHead (B=8, S=2048, D=4096).

v3 strategy (8 NeuronCores, batch-parallel + head-parallel):
  - Host folds weight-only math:  U = wk^T (probe@wq + bq) / sqrt(DH)
    (removes wq/wk and the U AllGather);  w2rin = mlp_w2 @ rin_w[pool]
    ((attn_out+h) is consumed only through rin_w -> the 16 MiB w2 stream
    becomes 1 MiB and the mlp AllReduce becomes the 8 KiB x0 AllReduce);
    LN affine gains fold into w1 / blk_w1 rows (y_aff@W = y_core@(g*W) +
    (b@W folded into the bias)).
  - Scores stream llm^T in fp8 e3m4 (softmax washes the quantization to
    ~0.2% on attention weights); pooled streams llm natural in f16.
  - All m=8 matmuls 4-way column-tiled (tile_position, measured 2.35x).
  - Pooled runs in two D-halves with two pipelined AllToAlls; ctx
    consumes each half as it lands.  x0 partials (attn_part@rin_pool,
    computed during the attn AllReduce) collapse into one 8 KiB AR.
  - Biases enter PSUM via 128-row replicated bias tile (ones/128) so all
    matmuls keep the (128,32) PE tiling mode.
  - Rings: scalar = llm streams + wo + odd w1; sync = rin_pool + even w1
    + w2rin + tail weights; gpsimd = smalls, wv, collective bounces
    (with f16<->f32 casts on the attn AllReduce wire).
"""

import numpy as np
import sys

if "/opt/trn_rl_repo" not in sys.path:
    sys.path.insert(0, "/opt/trn_rl_repo")

import ml_dtypes
import concourse.bass as bass
import concourse.tile as tile
from concourse import bacc, mybir
from concourse.masks import make_identity
from concourse.bass_utils import run_bass_kernel_spmd

F32 = mybir.dt.float32
F16 = mybir.dt.float16
F8 = mybir.dt.float8e3
NP8 = ml_dtypes.float8_e3m4
AF = mybir.ActivationFunctionType
ALU = mybir.AluOpType

B, S, D = 8, 2048, 4096
H, AD, TD, HID, NBLK = 8, 7, 32, 256, 3
DH = D // H
NC = 8
P = 128
SC = S // P            # 16
DC = D // P            # 32
HD2 = D // 2           # 2048
F1S = 4 * D // NC      # 2048
FC = F1S // P          # 16
HC = HID // P          # 2
SU = 2048.0
RSQRT_DH = 1.0 / float(np.sqrt(DH))
TWO_PI = 2.0 * float(np.pi)

BO_O = 0
B1_O = 4096
BV_O = 6144
BB1_O = 6656
BB2_O = 9728
BREP_N = 10496


def _bcast(src_ap, nparts):
    ap = src_ap
    assert ap.shape[0] == 1, ap.shape
    return bass.AP(tensor=ap.tensor, offset=ap.offset,
                   ap=[[0, nparts]] + [list(x) for x in ap.ap[1:]])


def build_program():
    nc = bacc.Bacc("TRN2", target_bir_lowering=False, debug=False,
                   num_devices=NC)
    t = {}

    def din(name, shape, dtype=F32):
        t[name] = nc.dram_tensor(name, shape, dtype, kind="ExternalInput")

    din("llm16", [S, D], F16)
    din("llmT8", [D, S], F8)
    din("U8r", [P, DC, H], F8)
    din("wv16", [8, P, 4, DH], F16)
    din("wo16", [DH, D], F16)
    din("w116", [16, P, 2, F1S], F16)
    din("w2rin16", [P, FC, HID], F16)
    din("rp16", [P, DC, HID], F16)
    din("brep", [1, BREP_N], F16)
    din("four_w2", [TD, 1]); din("phase2", [TD, 1])
    din("timeT", [1, B]); din("naT", [AD, B], F16)
    din("cw1", [TD, 2 * TD], F16); din("cb1c", [2 * TD, 1])
    din("cw2rin8", [2 * TD, HID], F16)
    din("rna8", [AD, HID], F16)
    din("rb8", [1, HID], F16)
    din("bw1a", [P, 3, 4 * HID], F16)
    din("bw1b", [P, 3, 4 * HID], F16)
    din("bw2a", [P, 12, HID], F16)
    din("bw2b", [P, 12, HID], F16)
    din("ow", [P, HC, AD], F16); din("out_bc", [1, AD])
    t["res"] = nc.dram_tensor("res", [B, AD], F32, kind="ExternalOutput")

    for hf in range(2):
        t[f"cc_pool_in{hf}"] = nc.dram_tensor(f"cc_pool_in{hf}", [H, HD2], F16)
        t[f"cc_pool_out{hf}"] = nc.dram_tensor(f"cc_pool_out{hf}", [B, HD2],
                                               F16)
    t["cc_attn_in"] = nc.dram_tensor("cc_attn_in", [B, D], F32)
    t["cc_attn_out"] = nc.dram_tensor("cc_attn_out", [B, D], F32,
                                      addr_space="Shared")
    t["cc_x0_in"] = nc.dram_tensor("cc_x0_in", [B, HID], F32)
    t["cc_x0_out"] = nc.dram_tensor("cc_x0_out", [B, HID], F32,
                                    addr_space="Shared")

    with tile.TileContext(nc) as tc:
        import contextlib
        with contextlib.ExitStack() as ctx:
            _build(nc, tc, t, ctx)
    nc.finalize()
    return nc


def _build(nc, tc, t, ctx):
    GROUPS = [list(range(NC))]

    singles = ctx.enter_context(tc.tile_pool(name="singles", bufs=1))
    lt8p = ctx.enter_context(tc.tile_pool(name="lt8p", bufs=2))
    ln16p = ctx.enter_context(tc.tile_pool(name="ln16p", bufs=6))
    natp = ctx.enter_context(tc.tile_pool(name="natp", bufs=2))
    wvp = ctx.enter_context(tc.tile_pool(name="wvp", bufs=8))
    w1p = ctx.enter_context(tc.tile_pool(name="w1p", bufs=3))
    psA = ctx.enter_context(tc.tile_pool(name="psA", bufs=2, space="PSUM"))
    psB = ctx.enter_context(tc.tile_pool(name="psB", bufs=2, space="PSUM"))
    psT8 = ctx.enter_context(tc.tile_pool(name="psT8", bufs=2, space="PSUM"))

    ident = singles.tile([P, P], F32)
    make_identity(nc, ident)
    ident16 = singles.tile([P, P], F16)
    nc.vector.tensor_copy(out=ident16[:], in_=ident[:])
    eps_sb = singles.tile([P, 1], F32)
    nc.vector.memset(eps_sb[:], 1e-5)
    ones8 = singles.tile([1, 8], F16)
    nc.vector.memset(ones8[:], 1.0)
    ones128 = singles.tile([P, 8], F16)
    nc.vector.memset(ones128[:], 1.0 / 128.0)

    def t_nat_to_T(src_nat, dst_T, nchunks, npart, uid, evict_eng=None,
                   c0=0):
        eng = evict_eng or nc.vector
        for c in range(nchunks):
            ps = psT8.tile([P, 16], F16, tag="tp16", name=f"tp_{uid}_{c}")
            nc.tensor.transpose(ps[:, :npart], src_nat[:, c * P:(c + 1) * P],
                                ident16[:npart, :npart])
            if eng is nc.scalar:
                nc.scalar.activation(out=dst_T[:, c0 + c, :],
                                     in_=ps[:, :npart], func=AF.Identity)
            else:
                eng.tensor_copy(out=dst_T[:, c0 + c, :], in_=ps[:, :npart])

    def bias_rep(ps_slice, col0, n_total, tp, stop):
        nchn = (n_total + 511) // 512
        for n in range(nchn):
            w = min(512, n_total - n * 512)
            nc.tensor.matmul(
                ps_slice[:, n * 512:n * 512 + w], ones128[:, :B],
                brep_sb[:, col0 + n * 512:col0 + n * 512 + w],
                start=False, stop=stop, tile_position=tp)

    def quad_sum(dst, ps, n):
        nc.vector.tensor_copy(out=dst, in_=ps[0:B, :n])
        nc.vector.tensor_add(out=dst, in0=dst, in1=ps[32:32 + B, :n])
        nc.vector.tensor_add(out=dst, in0=dst, in1=ps[64:64 + B, :n])
        nc.vector.tensor_add(out=dst, in0=dst, in1=ps[96:96 + B, :n])

    def layernorm_nat(x_nat, npart, n, y_nat, uid):
        nsub = max(1, n // 512)
        st = singles.tile([npart, nsub, nc.vector.BN_STATS_DIM], F32,
                          name=f"lnst_{uid}")
        xg = x_nat.rearrange("p (a b) -> p a b", a=nsub)
        for g in range(nsub):
            nc.vector.bn_stats(out=st[:, g, :], in_=xg[:, g, :])
        mv = singles.tile([npart, nc.vector.BN_AGGR_DIM], F32,
                          name=f"lnmv_{uid}")
        nc.vector.bn_aggr(out=mv[:], in_=st[:])
        std = singles.tile([npart, 1], F32, name=f"lnsd_{uid}")
        nc.scalar.activation(out=std[:], in_=mv[:, 1:2], func=AF.Sqrt,
                             bias=eps_sb[:npart, :])
        nc.vector.reciprocal(out=std[:], in_=std[:])
        nc.vector.tensor_scalar(out=y_nat, in0=x_nat, scalar1=mv[:, 0:1],
                                scalar2=std[:], op0=ALU.subtract, op1=ALU.mult)

    # ===== STEP 0: U + rp on sync; llmT8 stream hoisted on scalar; smalls
    # on gpsimd (cond inputs first, bulky brep last).
    u8_sb = singles.tile([P, DC, H], F8)
    nc.sync.dma_start(out=u8_sb[:], in_=t["U8r"][:])
    rp_sb = singles.tile([P, DC, HID], F16)
    nc.sync.dma_start(out=rp_sb[:], in_=t["rp16"][:])

    lt_tiles = []
    for g in range(8):
        lt = lt8p.tile([P, 4, S], F8, tag="lt8", name=f"lt8_{g}")
        nc.scalar.dma_start(
            out=lt[:],
            in_=t["llmT8"][g * 512:(g + 1) * 512, :].rearrange(
                "(c p) s -> p c s", p=P))
        lt_tiles.append(lt)

    fw_sb = singles.tile([TD, 1], F32)
    nc.gpsimd.dma_start(out=fw_sb[:], in_=t["four_w2"][:])
    ph_sb = singles.tile([TD, 1], F32)
    nc.gpsimd.dma_start(out=ph_sb[:], in_=t["phase2"][:])
    tb32 = singles.tile([TD, B], F32)
    nc.gpsimd.dma_start(out=tb32[:], in_=_bcast(t["timeT"][:], TD))
    cw1_sb = singles.tile([TD, 2 * TD], F16)
    nc.gpsimd.dma_start(out=cw1_sb[:], in_=t["cw1"][:])
    cb1_sb = singles.tile([2 * TD, 1], F32)
    nc.gpsimd.dma_start(out=cb1_sb[:], in_=t["cb1c"][:])
    cwr_sb = singles.tile([2 * TD, HID], F16)
    nc.gpsimd.dma_start(out=cwr_sb[:], in_=t["cw2rin8"][:])
    naT_sb = singles.tile([AD, B], F16)
    nc.gpsimd.dma_start(out=naT_sb[:], in_=t["naT"][:])
    rna_sb = singles.tile([AD, HID], F16)
    nc.gpsimd.dma_start(out=rna_sb[:], in_=t["rna8"][:])
    rb_sb = singles.tile([1, HID], F16)
    nc.gpsimd.dma_start(out=rb_sb[:], in_=t["rb8"][:])
    wv_tiles = []
    for g in range(8):
        wt = wvp.tile([P, 4, DH], F16, tag="wv", name=f"wv_{g}")
        nc.gpsimd.dma_start(out=wt[:], in_=t["wv16"][g])
        wv_tiles.append(wt)
    brep_sb = singles.tile([P, BREP_N], F16)
    nc.gpsimd.dma_start(out=brep_sb[:], in_=_bcast(t["brep"][:], P))
    ow_sb = singles.tile([P, HC, AD], F16)
    nc.gpsimd.dma_start(out=ow_sb[:], in_=t["ow"][:])
    ob_bc = singles.tile([B, AD], F32)
    nc.gpsimd.dma_start(out=ob_bc[:], in_=_bcast(t["out_bc"][:], B))

    # ===== STEP 1: scoresT = (U*SU)^T @ llmT  [fp8, tiles-over-n]
    ps_sc = psA.tile([P, 1024], F32, tag="psA", name="ps_sc")
    for g in range(8):
        for cc in range(4):
            c = 4 * g + cc
            for j in range(4):
                nc.tensor.matmul(
                    ps_sc[32 * j:32 * j + H, 0:512],
                    u8_sb[:, c, :], lt_tiles[g][:, cc, 512 * j:512 * (j + 1)],
                    start=(c == 0), stop=(c == DC - 1),
                    tile_position=(0, 32 * j))

    p_nat = natp.tile([H, S], F16, tag="nat8", name="p_nat")
    for j in range(4):
        nc.scalar.activation(out=p_nat[:, 512 * j:512 * (j + 1)],
                             in_=ps_sc[32 * j:32 * j + H, 0:512], func=AF.Exp,
                             scale=1.0 / SU)
    den = singles.tile([H, 1], F32)
    nc.vector.reduce_sum(out=den[:], in_=p_nat[:], axis=mybir.AxisListType.X)
    nc.vector.reciprocal(out=den[:], in_=den[:])
    pT = singles.tile([P, SC, H], F16)
    t_nat_to_T(p_nat, pT, SC, H, "p")

    # ===== STEP 2: pooled = pT^T @ llm / den.  D-half row-pair 1 MiB
    # tiles, all on the fast scalar ring; AllToAll per half, pipelined
    # with the second half's stream.
    poolh = []
    for hf in range(2):
        ps_pool = psA.tile([P, 1024], F32, tag="psA", name=f"ps_pool{hf}")
        for tt in range(8):
            lt = ln16p.tile([P, 2, HD2], F16, tag="ln16",
                            name=f"ln16_{hf}_{tt}")
            nc.scalar.dma_start(
                out=lt[:],
                in_=t["llm16"][tt * 256:(tt + 1) * 256,
                               hf * HD2:(hf + 1) * HD2].rearrange(
                                   "(a p) d -> p a d", p=P))
            for cc in range(2):
                c = 2 * tt + cc
                for j in range(4):
                    nc.tensor.matmul(
                        ps_pool[32 * j:32 * j + H, 0:512],
                        pT[:, c, :], lt[:, cc, 512 * j:512 * (j + 1)],
                        start=(c == 0), stop=(c == SC - 1),
                        tile_position=(0, 32 * j))
        pooled = natp.tile([H, HD2], F16, tag="nat8", name=f"pooled{hf}")
        for j in range(4):
            nc.vector.tensor_scalar(
                out=pooled[:, 512 * j:512 * (j + 1)],
                in0=ps_pool[32 * j:32 * j + H, 0:512],
                scalar1=den[:], scalar2=None, op0=ALU.mult)
        nc.gpsimd.dma_start(out=t[f"cc_pool_in{hf}"][:], in_=pooled[:])
        nc.gpsimd.collective_compute(
            "AllToAll", ALU.bypass, replica_groups=GROUPS,
            ins=[t[f"cc_pool_in{hf}"][:].opt()],
            outs=[t[f"cc_pool_out{hf}"][:].opt()])
        ph_t = natp.tile([B, HD2], F16, tag="nat8", name=f"poolh{hf}")
        nc.gpsimd.dma_start(out=ph_t[:], in_=t[f"cc_pool_out{hf}"][:])
        poolh.append(ph_t)

    # ---- cond path (off critical path; PE slots in while streams run)
    fu = singles.tile([TD, B], F32)
    nc.vector.tensor_scalar_mul(out=fu[:], in0=tb32[:], scalar1=fw_sb[:])
    fi = singles.tile([TD, B], mybir.dt.int32)
    nc.vector.tensor_copy(out=fi[:], in_=fu[:])
    fif = singles.tile([TD, B], F32)
    nc.vector.tensor_copy(out=fif[:], in_=fi[:])
    nc.vector.tensor_sub(out=fu[:], in0=fu[:], in1=fif[:])
    ffT = singles.tile([TD, B], F16)
    nc.scalar.activation(out=ffT[:], in_=fu[:], func=AF.Sin,
                         scale=TWO_PI, bias=ph_sb[:])
    ps_c1 = psB.tile([P, 512], F32, tag="psB", name="ps_c1")
    nc.tensor.matmul(ps_c1[:2 * TD, :B], cw1_sb[:], ffT[:], start=True,
                     stop=True)
    c1 = singles.tile([2 * TD, B], F16)
    nc.scalar.activation(out=c1[:], in_=ps_c1[:2 * TD, :B], func=AF.Silu,
                         bias=cb1_sb[:])
    ps_e = psB.tile([P, 512], F32, tag="psB", name="ps_e")
    nc.tensor.matmul(ps_e[:B, :HID], c1[:], cwr_sb[:], start=True, stop=False)
    nc.tensor.matmul(ps_e[:B, :HID], naT_sb[:], rna_sb[:], start=False,
                     stop=False)
    nc.tensor.matmul(ps_e[:B, :HID], ones8[:, :B], rb_sb[:], start=False,
                     stop=True)
    x0_early = singles.tile([B, HID], F32)
    nc.vector.tensor_copy(out=x0_early[:], in_=ps_e[:B, :HID])

    # ===== STEP 3: ctx = poolh @ wv + bv  [tiles-over-k, half-pipelined]
    poolhT = singles.tile([P, DC, B], F16)
    ps_cx = psB.tile([P, 512], F32, tag="psB", name="ps_cx")
    for hf in range(2):
        t_nat_to_T(poolh[hf], poolhT, SC, B, f"ph{hf}", c0=hf * SC)
        for g in range(4 * hf, 4 * hf + 4):
            for cc in range(4):
                c = 4 * g + cc
                j = c % 4
                nc.tensor.matmul(ps_cx[32 * j:32 * j + B, :],
                                 poolhT[:, c, :], wv_tiles[g][:, cc, :],
                                 start=(c < 4),
                                 stop=(c >= DC - 4 and j != 0),
                                 tile_position=(0, 32 * j))
    bias_rep(ps_cx[0:B, :], BV_O, DH, (0, 0), stop=True)
    ctx_nat = natp.tile([B, DH], F16, tag="nat8", name="ctx_nat")
    quad_sum(ctx_nat[:], ps_cx, DH)
    ctxT = singles.tile([P, DH // P, B], F16)
    t_nat_to_T(ctx_nat, ctxT, DH // P, B, "cx")

    # ===== STEP 4: attn partial = ctx @ wo + bo/8 ; AllReduce (f32 wire)
    wo_tiles = []
    for c in range(4):
        for hf in range(2):
            wt = wvp.tile([P, HD2], F16, tag="wv", name=f"wo_{c}_{hf}")
            nc.gpsimd.dma_start(
                out=wt[:],
                in_=t["wo16"][c * P:(c + 1) * P, hf * HD2:(hf + 1) * HD2])
            wo_tiles.append(wt)
    ps_at = psA.tile([P, 1024], F32, tag="psA", name="ps_at")
    for c in range(4):
        for j in range(4):
            for u in range(2):
                n0 = 1024 * j + 512 * u
                nc.tensor.matmul(
                    ps_at[32 * j:32 * j + B, 512 * u:512 * (u + 1)],
                    ctxT[:, c, :],
                    wo_tiles[2 * c + n0 // HD2][:, n0 % HD2:n0 % HD2 + 512],
                    start=(c == 0), stop=False,
                    tile_position=(0, 32 * j))
    for j in range(4):
        bias_rep(ps_at[32 * j:32 * j + B, :], BO_O + 1024 * j, 1024,
                 (0, 32 * j), stop=True)
    attn_part = natp.tile([B, D], F16, tag="natD", name="attn_part")
    for j in range(4):
        nc.scalar.activation(out=attn_part[:, 1024 * j:1024 * (j + 1)],
                             in_=ps_at[32 * j:32 * j + B, :],
                             func=AF.Identity)
    # w1 chunks 0..15 stream on sync (window fills from T~10); chunks
    # 16..31 ride the freed wv pool slots on gpsimd during the AllReduce.
    w1_tiles = []
    for g in range(8):
        wt = w1p.tile([P, 2, F1S], F16, tag="w1", name=f"w1_{g}")
        nc.scalar.dma_start(out=wt[:], in_=t["w116"][g])
        w1_tiles.append(wt)
    nc.gpsimd.dma_start(out=t["cc_attn_in"][:], in_=attn_part[:])
    nc.gpsimd.collective_compute(
        "AllReduce", ALU.add, replica_groups=GROUPS,
        ins=[t["cc_attn_in"][:].opt()], outs=[t["cc_attn_out"][:].opt()])
    attn_nat = natp.tile([B, D], F16, tag="natD", name="attn_nat")
    nc.gpsimd.dma_start(out=attn_nat[:], in_=t["cc_attn_out"][:])
    w1g_tiles = []
    for k in range(16):
        wt = wvp.tile([P, F1S], F16, tag="wv", name=f"w1g_{k}")
        nc.gpsimd.dma_start(out=wt[:], in_=t["w116"][8 + k // 2, :, k % 2, :])
        w1g_tiles.append(wt)

    # overlapped with the AllReduce: x0 += attn_partial @ rin_pool
    apT = singles.tile([P, DC, B], F16)
    t_nat_to_T(attn_part, apT, DC, B, "ap")
    ps_xa = psB.tile([P, 512], F32, tag="psB", name="ps_xa")
    for c in range(DC):
        j = c % 4
        nc.tensor.matmul(ps_xa[32 * j:32 * j + B, :HID], apT[:, c, :],
                         rp_sb[:, c, :], start=(c < 4), stop=(c >= DC - 4),
                         tile_position=(0, 32 * j))
    xa_nat = singles.tile([B, HID], F32)
    quad_sum(xa_nat[:], ps_xa, HID)
    nc.vector.tensor_add(out=x0_early[:], in0=x0_early[:], in1=xa_nat[:])

    # ===== STEP 5: y = LN(attn) (affine folded into w1) ; mm1
    y_nat = natp.tile([B, D], F16, tag="natD", name="y_nat")
    layernorm_nat(attn_nat[:], B, D, y_nat[:], "ln0")
    yT = singles.tile([P, DC, B], F16)
    t_nat_to_T(y_nat, yT, DC, B, "y")

    ps_h1 = psA.tile([P, 1024], F32, tag="psA", name="ps_h1")
    for c in range(DC):
        src = (w1_tiles[c // 2][:, c % 2, :] if c < 16
               else w1g_tiles[c - 16][:, :])
        for j in range(4):
            nc.tensor.matmul(
                ps_h1[32 * j:32 * j + B, 0:512],
                yT[:, c, :], src[:, 512 * j:512 * (j + 1)],
                start=(c == 0), stop=False,
                tile_position=(0, 32 * j))
    for j in range(4):
        bias_rep(ps_h1[32 * j:32 * j + B, 0:512], B1_O + 512 * j, 512,
                 (0, 32 * j), stop=True)
    g_nat = natp.tile([B, F1S], F16, tag="nat8", name="g_nat")
    for j in range(4):
        nc.scalar.activation(out=g_nat[:, 512 * j:512 * (j + 1)],
                             in_=ps_h1[32 * j:32 * j + B, 0:512],
                             func=AF.Gelu)
    gT = singles.tile([P, FC, B], F16)
    t_nat_to_T(g_nat, gT, FC, B, "g")

    # ===== STEP 6: x0 += g1 @ w2rin ; AllReduce(x0)
    w2r_sb = w1p.tile([P, FC, HID], F16, tag="w2r", bufs=1, name="w2r")
    nc.sync.dma_start(out=w2r_sb[:], in_=t["w2rin16"][:])
    ps_x0 = psB.tile([P, 512], F32, tag="psB", name="ps_x0")
    for c in range(FC):
        j = c % 4
        nc.tensor.matmul(ps_x0[32 * j:32 * j + B, :HID], gT[:, c, :],
                         w2r_sb[:, c, :], start=(c < 4), stop=(c >= FC - 4),
                         tile_position=(0, 32 * j))
    xg_nat = singles.tile([B, HID], F32)
    quad_sum(xg_nat[:], ps_x0, HID)
    nc.vector.tensor_add(out=x0_early[:], in0=x0_early[:], in1=xg_nat[:])
    nc.gpsimd.dma_start(out=t["cc_x0_in"][:], in_=x0_early[:])
    nc.gpsimd.collective_compute(
        "AllReduce", ALU.add, replica_groups=GROUPS,
        ins=[t["cc_x0_in"][:].opt()], outs=[t["cc_x0_out"][:].opt()])

    # ===== STEP 7: diffusion tail (replicated; blk LN affine folded)
    bw1a = w1p.tile([P, 3, 4 * HID], F16, tag="w1", name="bw1a")
    nc.sync.dma_start(out=bw1a[:], in_=t["bw1a"][:])
    bw1b = w1p.tile([P, 3, 4 * HID], F16, tag="w1", name="bw1b")
    nc.sync.dma_start(out=bw1b[:], in_=t["bw1b"][:])
    bw2a = w1p.tile([P, 12, HID], F16, tag="w1", name="bw2a")
    nc.sync.dma_start(out=bw2a[:], in_=t["bw2a"][:])
    bw2b = w1p.tile([P, 12, HID], F16, tag="w1", name="bw2b")
    nc.sync.dma_start(out=bw2b[:], in_=t["bw2b"][:])

    x_nat = singles.tile([B, HID], F32)
    nc.gpsimd.dma_start(out=x_nat[:], in_=t["cc_x0_out"][:])

    for i in range(NBLK):
        xn = singles.tile([B, HID], F16, name=f"xn_{i}")
        layernorm_nat(x_nat[:], B, HID, xn[:], f"lnb{i}")
        xnT = singles.tile([P, HC, B], F16, name=f"xnT_{i}")
        t_nat_to_T(xn, xnT, HC, B, f"xn{i}")
        ps_bh = psB.tile([P, 512], F32, tag="psB", name=f"ps_bh_{i}")
        for j in range(4):
            for c in range(HC):
                f = 2 * i + c
                src = bw1a if f < 3 else bw1b
                nc.tensor.matmul(
                    ps_bh[32 * j:32 * j + B, 0:256],
                    xnT[:, c, :], src[:, f % 3, 256 * j:256 * (j + 1)],
                    start=(c == 0), stop=False,
                    tile_position=(0, 32 * j))
            bias_rep(ps_bh[32 * j:32 * j + B, 0:256],
                     BB1_O + 1024 * i + 256 * j, 256, (0, 32 * j), stop=True)
        hb = natp.tile([B, 4 * HID], F16, tag="nat8", name=f"hb_{i}")
        for j in range(4):
            nc.scalar.activation(out=hb[:, 256 * j:256 * (j + 1)],
                                 in_=ps_bh[32 * j:32 * j + B, 0:256],
                                 func=AF.Silu)
        hbT = singles.tile([P, 4 * HID // P, B], F16, name=f"hbT_{i}")
        t_nat_to_T(hb, hbT, 4 * HID // P, B, f"hb{i}")

        ps_bo = psB.tile([P, 512], F32, tag="psB", name=f"ps_bo_{i}")
        for c in range(4 * HID // P):
            j = c % 4
            f = 8 * i + c
            src = bw2a if f < 12 else bw2b
            nc.tensor.matmul(ps_bo[32 * j:32 * j + B, :HID], hbT[:, c, :],
                             src[:, f % 12, :],
                             start=(c < 4), stop=(c >= 4 and j != 0),
                             tile_position=(0, 32 * j))
        bias_rep(ps_bo[0:B, :HID], BB2_O + 256 * i, HID, (0, 0), stop=True)
        for q in range(4):
            nc.vector.tensor_add(out=x_nat[:], in0=x_nat[:],
                                 in1=ps_bo[32 * q:32 * q + B, :HID])

    xs = singles.tile([B, HID], F16)
    nc.scalar.activation(out=xs[:], in_=x_nat[:], func=AF.Silu)
    xsT = singles.tile([P, HC, B], F16)
    t_nat_to_T(xs, xsT, HC, B, "xs")
    ps_o = psB.tile([P, 512], F32, tag="psB", name="ps_o")
    for c in range(HC):
        nc.tensor.matmul(ps_o[:B, :AD], xsT[:, c, :], ow_sb[:, c, :],
                         start=(c == 0), stop=(c == HC - 1))
    out_sb = singles.tile([B, AD], F32)
    nc.vector.tensor_add(out=out_sb[:], in0=ps_o[:B, :AD], in1=ob_bc[:])
    nc.sync.dma_start(out=t["res"][:], in_=out_sb[:])


_CACHED_NC = None


def _get_nc():
    global _CACHED_NC
    if _CACHED_NC is None:
        _CACHED_NC = build_program()
    return _CACHED_NC


def _prep_in_maps(inputs):
    f32 = np.float32
    f16 = np.float16
    llm_full = np.asarray(inputs["llm_output"], dtype=f32)
    wq = np.asarray(inputs["wq"], f32); wk = np.asarray(inputs["wk"], f32)
    wv = np.asarray(inputs["wv"], f32); wo = np.asarray(inputs["wo"], f32)
    bq = np.asarray(inputs["bq"], f32); bv = np.asarray(inputs["bv"], f32)
    bo = np.asarray(inputs["bo"], f32)
    ln_g = np.asarray(inputs["ln_g"], f32)
    ln_b = np.asarray(inputs["ln_b"], f32)
    w1 = np.asarray(inputs["mlp_w1"], f32); b1 = np.asarray(inputs["mlp_b1"], f32)
    w2 = np.asarray(inputs["mlp_w2"], f32); b2 = np.asarray(inputs["mlp_b2"], f32)
    rin_w = np.asarray(inputs["rin_w"], f32)
    rin_b = np.asarray(inputs["rin_b"], f32)
    probe = np.asarray(inputs["probe"], f32).reshape(D)
    cw2 = np.asarray(inputs["cond_w2"], f32)
    cb2 = np.asarray(inputs["cond_b2"], f32)
    blk_g = np.asarray(inputs["blk_ln_g"], f32)
    blk_b = np.asarray(inputs["blk_ln_b"], f32)
    blk_w1 = np.asarray(inputs["blk_w1"], f32)
    blk_w2 = np.asarray(inputs["blk_w2"], f32)
    blk_b1 = np.asarray(inputs["blk_b1"], f32)
    blk_b2 = np.asarray(inputs["blk_b2"], f32)

    # ---- weight-only folds ----
    q = (probe @ wq + bq) * RSQRT_DH
    U = np.zeros((D, H), f32)
    for h in range(H):
        U[:, h] = wk[:, h * DH:(h + 1) * DH] @ q[h * DH:(h + 1) * DH]
    U8 = (U * SU).astype(NP8)
    rin_cond = rin_w[0:TD]
    rin_pool = np.ascontiguousarray(rin_w[TD:TD + D])
    rin_na = rin_w[TD + D:]
    w2rin = w2 @ rin_pool
    cw2rin = cw2 @ rin_cond
    rb_fold = (rin_b + b2 @ rin_pool + cb2 @ rin_cond) / NC
    # LN affine folds: y_aff @ W = y_core @ (g*W) + b@W
    w1_aff = ln_g[:, None] * w1              # (D, 4D)
    b1_aff = b1 + ln_b @ w1                  # (4D,)
    bw1_aff = blk_g[:, :, None] * blk_w1     # (3, HID, 4HID)
    bb1_aff = blk_b1 + np.einsum('ih,ihf->if', blk_b, blk_w1)

    def ptile(m, c_per_g):
        K, N = m.shape
        G = K // (P * c_per_g)
        r = np.ascontiguousarray(
            m.reshape(G, c_per_g, P, N).transpose(0, 2, 1, 3))
        return r if G > 1 else r[0]

    shared = {
        "rp16": np.ascontiguousarray(
            rin_pool.reshape(DC, P, HID).transpose(1, 0, 2)).astype(f16),
        "four_w2": np.concatenate(
            [np.asarray(inputs["four_w"], f32).reshape(TD // 2, 1)] * 2),
        "phase2": np.concatenate(
            [np.full((TD // 2, 1), np.pi / 2, f32),
             np.zeros((TD // 2, 1), f32)]),
        "timeT": np.ascontiguousarray(np.asarray(inputs["time"], f32).T),
        "naT": np.ascontiguousarray(
            np.asarray(inputs["noisy_actions"], f32).T).astype(f16),
        "cw1": np.asarray(inputs["cond_w1"], f32).astype(f16),
        "cb1c": np.asarray(inputs["cond_b1"], f32).reshape(-1, 1),
        "cw2rin8": (cw2rin / NC).astype(f16),
        "rna8": (rin_na / NC).astype(f16),
        "rb8": rb_fold.astype(f16).reshape(1, HID),
        "bw1a": np.ascontiguousarray(
            bw1_aff.reshape(NBLK * HC, P, 4 * HID)[0:3].transpose(1, 0, 2)
        ).astype(f16),
        "bw1b": np.ascontiguousarray(
            bw1_aff.reshape(NBLK * HC, P, 4 * HID)[3:6].transpose(1, 0, 2)
        ).astype(f16),
        "bw2a": np.ascontiguousarray(
            blk_w2.reshape(NBLK * 8, P, HID)[0:12].transpose(1, 0, 2)
        ).astype(f16),
        "bw2b": np.ascontiguousarray(
            blk_w2.reshape(NBLK * 8, P, HID)[12:24].transpose(1, 0, 2)
        ).astype(f16),
        "ow": np.ascontiguousarray(
            np.asarray(inputs["out_w"], f32).reshape(HC, P, AD)
            .transpose(1, 0, 2)).astype(f16),
        "out_bc": np.asarray(inputs["out_b"], f32).reshape(1, AD),
        "U8r": np.ascontiguousarray(U8.reshape(DC, P, H).transpose(1, 0, 2)),
    }

    in_maps = []
    for i in range(NC):
        hb_ = slice(i * DH, (i + 1) * DH)
        fb = slice(i * F1S, (i + 1) * F1S)
        m = dict(shared)
        m["llm16"] = llm_full[i].astype(f16)
        m["llmT8"] = np.ascontiguousarray(llm_full[i].T).astype(NP8)
        m["wv16"] = ptile(np.ascontiguousarray(wv[:, hb_]), 4).astype(f16)
        m["wo16"] = np.ascontiguousarray(wo[hb_, :]).astype(f16)
        m["w116"] = ptile(np.ascontiguousarray(w1_aff[:, fb]), 2).astype(f16)
        m["w2rin16"] = ptile(np.ascontiguousarray(w2rin[fb]), FC).astype(f16)
        brep = np.zeros((1, BREP_N), f16)
        brep[0, BO_O:BO_O + D] = (bo / NC).astype(f16)
        brep[0, B1_O:B1_O + F1S] = b1_aff[fb].astype(f16)
        brep[0, BV_O:BV_O + DH] = bv[hb_].astype(f16)
        brep[0, BB1_O:BB1_O + NBLK * 4 * HID] = bb1_aff.reshape(-1).astype(f16)
        brep[0, BB2_O:BB2_O + NBLK * HID] = blk_b2.reshape(-1).astype(f16)
        m["brep"] = brep
        in_maps.append(m)
    return in_maps


def kernel(**inputs):
    nc = _get_nc()
    in_maps = _prep_in_maps(inputs)
    r = run_bass_kernel_spmd(nc, in_maps, core_ids=list(range(NC)))
    return np.ascontiguousarray(r.results[0]["res"]).astype(np.float32)


def run_traced(**inputs):
    nc = _get_nc()
    in_maps = _prep_in_maps(inputs)
    r = run_bass_kernel_spmd(nc, in_maps, core_ids=list(range(NC)), trace=True)
    return np.ascontiguousarray(r.results[0]["res"]).astype(np.float32), r


# revision 29
# speedup vs baseline: 1.1210x; 1.0073x over previous
"""Trainium2 Bass kernel for nn_DiffusionActionHead (B=8, S=2048, D=4096).

v3 strategy (8 NeuronCores, batch-parallel + head-parallel):
  - Host folds weight-only math:  U = wk^T (probe@wq + bq) / sqrt(DH)
    (removes wq/wk and the U AllGather);  w2rin = mlp_w2 @ rin_w[pool]
    ((attn_out+h) is consumed only through rin_w -> the 16 MiB w2 stream
    becomes 1 MiB and the mlp AllReduce becomes the 8 KiB x0 AllReduce);
    LN affine gains fold into w1 / blk_w1 rows (y_aff@W = y_core@(g*W) +
    (b@W folded into the bias)).
  - Scores stream llm^T in fp8 e3m4 (softmax washes the quantization to
    ~0.2% on attention weights); pooled streams llm natural in f16.
  - All m=8 matmuls 4-way column-tiled (tile_position, measured 2.35x).
  - Pooled runs in two D-halves with two pipelined AllToAlls; ctx
    consumes each half as it lands.  x0 partials (attn_part@rin_pool,
    computed during the attn AllReduce) collapse into one 8 KiB AR.
  - Biases enter PSUM via 128-row replicated bias tile (ones/128) so all
    matmuls keep the (128,32) PE tiling mode.
  - Rings: scalar = llm streams + wo + odd w1; sync = rin_pool + even w1
    + w2rin + tail weights; gpsimd = smalls, wv, collective bounces
    (with f16<->f32 casts on the attn AllReduce wire).
"""

import numpy as np
import sys

if "/opt/trn_rl_repo" not in sys.path:
    sys.path.insert(0, "/opt/trn_rl_repo")

import ml_dtypes
import concourse.bass as bass
import concourse.tile as tile
from concourse import bacc, mybir
from concourse.masks import make_identity
from concourse.bass_utils import run_bass_kernel_spmd

F32 = mybir.dt.float32
F16 = mybir.dt.float16
F8 = mybir.dt.float8e3
NP8 = ml_dtypes.float8_e3m4
AF = mybir.ActivationFunctionType
ALU = mybir.AluOpType

B, S, D = 8, 2048, 4096
H, AD, TD, HID, NBLK = 8, 7, 32, 256, 3
DH = D // H
NC = 8
P = 128
SC = S // P            # 16
DC = D // P            # 32
HD2 = D // 2           # 2048
F1S = 4 * D // NC      # 2048
FC = F1S // P          # 16
HC = HID // P          # 2
SU = 2048.0
RSQRT_DH = 1.0 / float(np.sqrt(DH))
TWO_PI = 2.0 * float(np.pi)

BO_O = 0
B1_O = 4096
BV_O = 6144
BB1_O = 6656
BB2_O = 9728
BREP_N = 10496


def _bcast(src_ap, nparts):
    ap = src_ap
    assert ap.shape[0] == 1, ap.shape
    return bass.AP(tensor=ap.tensor, offset=ap.offset,
                   ap=[[0, nparts]] + [list(x) for x in ap.ap[1:]])


def build_program():
    nc = bacc.Bacc("TRN2", target_bir_lowering=False, debug=False,
                   num_devices=NC)
    t = {}

    def din(name, shape, dtype=F32):
        t[name] = nc.dram_tensor(name, shape, dtype, kind="ExternalInput")

    din("llm16", [S, D], F16)
    din("llmT8", [D, S], F8)
    din("U8r", [P, DC, H], F8)
    din("wv16", [8, P, 4, DH], F16)
    din("wo16", [DH, D], F16)
    din("w116", [16, P, 2, F1S], F16)
    din("w2rin16", [P, FC, HID], F16)
    din("rp16", [P, DC, HID], F16)
    din("brep", [1, BREP_N], F16)
    din("four_w2", [TD, 1]); din("phase2", [TD, 1])
    din("timeT", [1, B]); din("naT", [AD, B], F16)
    din("cw1", [TD, 2 * TD], F16); din("cb1c", [2 * TD, 1])
    din("cw2rin8", [2 * TD, HID], F16)
    din("rna8", [AD, HID], F16)
    din("rb8", [1, HID], F16)
    din("bw1a", [P, 3, 4 * HID], F16)
    din("bw1b", [P, 3, 4 * HID], F16)
    din("bw2a", [P, 12, HID], F16)
    din("bw2b", [P, 12, HID], F16)
    din("ow", [P, HC, AD], F16); din("out_bc", [1, AD])
    t["res"] = nc.dram_tensor("res", [B, AD], F32, kind="ExternalOutput")

    for hf in range(2):
        t[f"cc_pool_in{hf}"] = nc.dram_tensor(f"cc_pool_in{hf}", [H, HD2], F16)
        t[f"cc_pool_out{hf}"] = nc.dram_tensor(f"cc_pool_out{hf}", [B, HD2],
                                               F16)
    t["cc_attn_in"] = nc.dram_tensor("cc_attn_in", [B, D], F32)
    t["cc_attn_out"] = nc.dram_tensor("cc_attn_out", [B, D], F32,
                                      addr_space="Shared")
    t["cc_x0_in"] = nc.dram_tensor("cc_x0_in", [B, HID], F32)
    t["cc_x0_out"] = nc.dram_tensor("cc_x0_out", [B, HID], F32,
                                    addr_space="Shared")

    with tile.TileContext(nc) as tc:
        import contextlib
        with contextlib.ExitStack() as ctx:
            _build(nc, tc, t, ctx)
    nc.finalize()
    return nc


def _build(nc, tc, t, ctx):
    GROUPS = [list(range(NC))]

    singles = ctx.enter_context(tc.tile_pool(name="singles", bufs=1))
    lt8p = ctx.enter_context(tc.tile_pool(name="lt8p", bufs=2))
    ln16p = ctx.enter_context(tc.tile_pool(name="ln16p", bufs=6))
    natp = ctx.enter_context(tc.tile_pool(name="natp", bufs=2))
    wvp = ctx.enter_context(tc.tile_pool(name="wvp", bufs=8))
    w1p = ctx.enter_context(tc.tile_pool(name="w1p", bufs=3))
    psA = ctx.enter_context(tc.tile_pool(name="psA", bufs=2, space="PSUM"))
    psB = ctx.enter_context(tc.tile_pool(name="psB", bufs=2, space="PSUM"))
    psT8 = ctx.enter_context(tc.tile_pool(name="psT8", bufs=2, space="PSUM"))

    ident = singles.tile([P, P], F32)
    make_identity(nc, ident)
    ident16 = singles.tile([P, P], F16)
    nc.vector.tensor_copy(out=ident16[:], in_=ident[:])
    eps_sb = singles.tile([P, 1], F32)
    nc.vector.memset(eps_sb[:], 1e-5)
    ones8 = singles.tile([1, 8], F16)
    nc.vector.memset(ones8[:], 1.0)
    ones128 = singles.tile([P, 8], F16)
    nc.vector.memset(ones128[:], 1.0 / 128.0)

    def t_nat_to_T(src_nat, dst_T, nchunks, npart, uid, evict_eng=None,
                   c0=0):
        eng = evict_eng or nc.vector
        for c in range(nchunks):
            ps = psT8.tile([P, 16], F16, tag="tp16", name=f"tp_{uid}_{c}")
            nc.tensor.transpose(ps[:, :npart], src_nat[:, c * P:(c + 1) * P],
                                ident16[:npart, :npart])
            if eng is nc.scalar:
                nc.scalar.activation(out=dst_T[:, c0 + c, :],
                                     in_=ps[:, :npart], func=AF.Identity)
            else:
                eng.tensor_copy(out=dst_T[:, c0 + c, :], in_=ps[:, :npart])

    def bias_rep(ps_slice, col0, n_total, tp, stop):
        nchn = (n_total + 511) // 512
        for n in range(nchn):
            w = min(512, n_total - n * 512)
            nc.tensor.matmul(
                ps_slice[:, n * 512:n * 512 + w], ones128[:, :B],
                brep_sb[:, col0 + n * 512:col0 + n * 512 + w],
                start=False, stop=stop, tile_position=tp)

    def quad_sum(dst, ps, n):
        nc.vector.tensor_copy(out=dst, in_=ps[0:B, :n])
        nc.vector.tensor_add(out=dst, in0=dst, in1=ps[32:32 + B, :n])
        nc.vector.tensor_add(out=dst, in0=dst, in1=ps[64:64 + B, :n])
        nc.vector.tensor_add(out=dst, in0=dst, in1=ps[96:96 + B, :n])

    def layernorm_nat(x_nat, npart, n, y_nat, uid):
        nsub = max(1, n // 512)
        st = singles.tile([npart, nsub, nc.vector.BN_STATS_DIM], F32,
                          name=f"lnst_{uid}")
        xg = x_nat.rearrange("p (a b) -> p a b", a=nsub)
        for g in range(nsub):
            nc.vector.bn_stats(out=st[:, g, :], in_=xg[:, g, :])
        mv = singles.tile([npart, nc.vector.BN_AGGR_DIM], F32,
                          name=f"lnmv_{uid}")
        nc.vector.bn_aggr(out=mv[:], in_=st[:])
        std = singles.tile([npart, 1], F32, name=f"lnsd_{uid}")
        nc.scalar.activation(out=std[:], in_=mv[:, 1:2], func=AF.Sqrt,
                             bias=eps_sb[:npart, :])
        nc.vector.reciprocal(out=std[:], in_=std[:])
        nc.vector.tensor_scalar(out=y_nat, in0=x_nat, scalar1=mv[:, 0:1],
                                scalar2=std[:], op0=ALU.subtract, op1=ALU.mult)

    # ===== STEP 0: U + rp on sync; llmT8 stream hoisted on scalar; smalls
    # on gpsimd (cond inputs first, bulky brep last).
    u8_sb = singles.tile([P, DC, H], F8)
    nc.sync.dma_start(out=u8_sb[:], in_=t["U8r"][:])
    rp_sb = singles.tile([P, DC, HID], F16)
    nc.sync.dma_start(out=rp_sb[:], in_=t["rp16"][:])

    def _ln16_dma(hf, tt):
        lt = ln16p.tile([P, 2, HD2], F16, tag="ln16",
                        name=f"ln16_{hf}_{tt}")
        nc.scalar.dma_start(
            out=lt[:],
            in_=t["llm16"][tt * 256:(tt + 1) * 256,
                           hf * HD2:(hf + 1) * HD2].rearrange(
                               "(a p) d -> p a d", p=P))
        return lt

    lt_tiles = []
    ln16_tiles = {}
    for g in range(8):
        lt = lt8p.tile([P, 4, S], F8, tag="lt8", name=f"lt8_{g}")
        nc.scalar.dma_start(
            out=lt[:],
            in_=t["llmT8"][g * 512:(g + 1) * 512, :].rearrange(
                "(c p) s -> p c s", p=P))
        lt_tiles.append(lt)
        if g >= 5:  # prefill first pooled tiles so PE never waits at T45
            ln16_tiles[(0, g - 5)] = _ln16_dma(0, g - 5)

    fw_sb = singles.tile([TD, 1], F32)
    nc.gpsimd.dma_start(out=fw_sb[:], in_=t["four_w2"][:])
    ph_sb = singles.tile([TD, 1], F32)
    nc.gpsimd.dma_start(out=ph_sb[:], in_=t["phase2"][:])
    tb32 = singles.tile([TD, B], F32)
    nc.gpsimd.dma_start(out=tb32[:], in_=_bcast(t["timeT"][:], TD))
    cw1_sb = singles.tile([TD, 2 * TD], F16)
    nc.gpsimd.dma_start(out=cw1_sb[:], in_=t["cw1"][:])
    cb1_sb = singles.tile([2 * TD, 1], F32)
    nc.gpsimd.dma_start(out=cb1_sb[:], in_=t["cb1c"][:])
    cwr_sb = singles.tile([2 * TD, HID], F16)
    nc.gpsimd.dma_start(out=cwr_sb[:], in_=t["cw2rin8"][:])
    naT_sb = singles.tile([AD, B], F16)
    nc.gpsimd.dma_start(out=naT_sb[:], in_=t["naT"][:])
    rna_sb = singles.tile([AD, HID], F16)
    nc.gpsimd.dma_start(out=rna_sb[:], in_=t["rna8"][:])
    rb_sb = singles.tile([1, HID], F16)
    nc.gpsimd.dma_start(out=rb_sb[:], in_=t["rb8"][:])
    wv_tiles = []
    for g in range(8):
        wt = wvp.tile([P, 4, DH], F16, tag="wv", name=f"wv_{g}")
        nc.gpsimd.dma_start(out=wt[:], in_=t["wv16"][g])
        wv_tiles.append(wt)
    brep_sb = singles.tile([P, BREP_N], F16)
    nc.gpsimd.dma_start(out=brep_sb[:], in_=_bcast(t["brep"][:], P))
    ow_sb = singles.tile([P, HC, AD], F16)
    nc.gpsimd.dma_start(out=ow_sb[:], in_=t["ow"][:])
    ob_bc = singles.tile([B, AD], F32)
    nc.gpsimd.dma_start(out=ob_bc[:], in_=_bcast(t["out_bc"][:], B))

    # ===== STEP 1: scoresT = (U*SU)^T @ llmT  [fp8, tiles-over-n]
    ps_sc = psA.tile([P, 1024], F32, tag="psA", name="ps_sc")
    for g in range(8):
        for cc in range(4):
            c = 4 * g + cc
            for j in range(4):
                nc.tensor.matmul(
                    ps_sc[32 * j:32 * j + H, 0:512],
                    u8_sb[:, c, :], lt_tiles[g][:, cc, 512 * j:512 * (j + 1)],
                    start=(c == 0), stop=(c == DC - 1),
                    tile_position=(0, 32 * j))

    p_nat = natp.tile([H, S], F16, tag="nat8", name="p_nat")
    for j in range(4):
        nc.scalar.activation(out=p_nat[:, 512 * j:512 * (j + 1)],
                             in_=ps_sc[32 * j:32 * j + H, 0:512], func=AF.Exp,
                             scale=1.0 / SU)
    den = singles.tile([H, 1], F32)
    nc.vector.reduce_sum(out=den[:], in_=p_nat[:], axis=mybir.AxisListType.X)
    nc.vector.reciprocal(out=den[:], in_=den[:])
    pT = singles.tile([P, SC, H], F16)
    t_nat_to_T(p_nat, pT, SC, H, "p")

    # ===== STEP 2: pooled = pT^T @ llm / den.  D-half row-pair 1 MiB
    # tiles, all on the fast scalar ring; AllToAll per half, pipelined
    # with the second half's stream.
    poolh = []
    for hf in range(2):
        ps_pool = psA.tile([P, 1024], F32, tag="psA", name=f"ps_pool{hf}")
        for tt in range(8):
            if (hf, tt) in ln16_tiles:
                lt = ln16_tiles[(hf, tt)]
            else:
                lt = _ln16_dma(hf, tt)
            for cc in range(2):
                c = 2 * tt + cc
                for j in range(4):
                    nc.tensor.matmul(
                        ps_pool[32 * j:32 * j + H, 0:512],
                        pT[:, c, :], lt[:, cc, 512 * j:512 * (j + 1)],
                        start=(c == 0), stop=(c == SC - 1),
                        tile_position=(0, 32 * j))
        pooled = natp.tile([H, HD2], F16, tag="nat8", name=f"pooled{hf}")
        for j in range(4):
            nc.vector.tensor_scalar(
                out=pooled[:, 512 * j:512 * (j + 1)],
                in0=ps_pool[32 * j:32 * j + H, 0:512],
                scalar1=den[:], scalar2=None, op0=ALU.mult)
        nc.gpsimd.dma_start(out=t[f"cc_pool_in{hf}"][:], in_=pooled[:])
        nc.gpsimd.collective_compute(
            "AllToAll", ALU.bypass, replica_groups=GROUPS,
            ins=[t[f"cc_pool_in{hf}"][:].opt()],
            outs=[t[f"cc_pool_out{hf}"][:].opt()])
        ph_t = natp.tile([B, HD2], F16, tag="nat8", name=f"poolh{hf}")
        nc.gpsimd.dma_start(out=ph_t[:], in_=t[f"cc_pool_out{hf}"][:])
        poolh.append(ph_t)

    # ---- cond path (off critical path; PE slots in while streams run)
    fu = singles.tile([TD, B], F32)
    nc.vector.tensor_scalar_mul(out=fu[:], in0=tb32[:], scalar1=fw_sb[:])
    fi = singles.tile([TD, B], mybir.dt.int32)
    nc.vector.tensor_copy(out=fi[:], in_=fu[:])
    fif = singles.tile([TD, B], F32)
    nc.vector.tensor_copy(out=fif[:], in_=fi[:])
    nc.vector.tensor_sub(out=fu[:], in0=fu[:], in1=fif[:])
    ffT = singles.tile([TD, B], F16)
    nc.scalar.activation(out=ffT[:], in_=fu[:], func=AF.Sin,
                         scale=TWO_PI, bias=ph_sb[:])
    ps_c1 = psB.tile([P, 512], F32, tag="psB", name="ps_c1")
    nc.tensor.matmul(ps_c1[:2 * TD, :B], cw1_sb[:], ffT[:], start=True,
                     stop=True)
    c1 = singles.tile([2 * TD, B], F16)
    nc.scalar.activation(out=c1[:], in_=ps_c1[:2 * TD, :B], func=AF.Silu,
                         bias=cb1_sb[:])
    ps_e = psB.tile([P, 512], F32, tag="psB", name="ps_e")
    nc.tensor.matmul(ps_e[:B, :HID], c1[:], cwr_sb[:], start=True, stop=False)
    nc.tensor.matmul(ps_e[:B, :HID], naT_sb[:], rna_sb[:], start=False,
                     stop=False)
    nc.tensor.matmul(ps_e[:B, :HID], ones8[:, :B], rb_sb[:], start=False,
                     stop=True)
    x0_early = singles.tile([B, HID], F32)
    nc.vector.tensor_copy(out=x0_early[:], in_=ps_e[:B, :HID])

    # ===== STEP 3: ctx = poolh @ wv + bv  [tiles-over-k, half-pipelined]
    poolhT = singles.tile([P, DC, B], F16)
    ps_cx = psB.tile([P, 512], F32, tag="psB", name="ps_cx")
    for hf in range(2):
        t_nat_to_T(poolh[hf], poolhT, SC, B, f"ph{hf}", c0=hf * SC)
        for g in range(4 * hf, 4 * hf + 4):
            for cc in range(4):
                c = 4 * g + cc
                j = c % 4
                nc.tensor.matmul(ps_cx[32 * j:32 * j + B, :],
                                 poolhT[:, c, :], wv_tiles[g][:, cc, :],
                                 start=(c < 4),
                                 stop=(c >= DC - 4 and j != 0),
                                 tile_position=(0, 32 * j))
    bias_rep(ps_cx[0:B, :], BV_O, DH, (0, 0), stop=True)
    ctx_nat = natp.tile([B, DH], F16, tag="nat8", name="ctx_nat")
    quad_sum(ctx_nat[:], ps_cx, DH)
    ctxT = singles.tile([P, DH // P, B], F16)
    t_nat_to_T(ctx_nat, ctxT, DH // P, B, "cx")

    # ===== STEP 4: attn partial = ctx @ wo + bo/8 ; AllReduce (f32 wire)
    wo_tiles = []
    for c in range(4):
        for hf in range(2):
            wt = wvp.tile([P, HD2], F16, tag="wv", name=f"wo_{c}_{hf}")
            nc.gpsimd.dma_start(
                out=wt[:],
                in_=t["wo16"][c * P:(c + 1) * P, hf * HD2:(hf + 1) * HD2])
            wo_tiles.append(wt)
    ps_at = psA.tile([P, 1024], F32, tag="psA", name="ps_at")
    for c in range(4):
        for j in range(4):
            for u in range(2):
                n0 = 1024 * j + 512 * u
                nc.tensor.matmul(
                    ps_at[32 * j:32 * j + B, 512 * u:512 * (u + 1)],
                    ctxT[:, c, :],
                    wo_tiles[2 * c + n0 // HD2][:, n0 % HD2:n0 % HD2 + 512],
                    start=(c == 0), stop=False,
                    tile_position=(0, 32 * j))
    for j in range(4):
        bias_rep(ps_at[32 * j:32 * j + B, :], BO_O + 1024 * j, 1024,
                 (0, 32 * j), stop=True)
    attn_part = natp.tile([B, D], F16, tag="natD", name="attn_part")
    for j in range(4):
        nc.scalar.activation(out=attn_part[:, 1024 * j:1024 * (j + 1)],
                             in_=ps_at[32 * j:32 * j + B, :],
                             func=AF.Identity)
    # w1 chunks 0..15 stream on sync (window fills from T~10); chunks
    # 16..31 ride the freed wv pool slots on gpsimd during the AllReduce.
    w1_tiles = []
    for g in range(8):
        wt = w1p.tile([P, 2, F1S], F16, tag="w1", name=f"w1_{g}")
        nc.scalar.dma_start(out=wt[:], in_=t["w116"][g])
        w1_tiles.append(wt)
    nc.gpsimd.dma_start(out=t["cc_attn_in"][:], in_=attn_part[:])
    nc.gpsimd.collective_compute(
        "AllReduce", ALU.add, replica_groups=GROUPS,
        ins=[t["cc_attn_in"][:].opt()], outs=[t["cc_attn_out"][:].opt()])
    attn_nat = natp.tile([B, D], F16, tag="natD", name="attn_nat")
    nc.gpsimd.dma_start(out=attn_nat[:], in_=t["cc_attn_out"][:])
    w1g_tiles = []
    for k in range(16):
        wt = wvp.tile([P, F1S], F16, tag="wv", name=f"w1g_{k}")
        nc.gpsimd.dma_start(out=wt[:], in_=t["w116"][8 + k // 2, :, k % 2, :])
        w1g_tiles.append(wt)

    # overlapped with the AllReduce: x0 += attn_partial @ rin_pool
    apT = singles.tile([P, DC, B], F16)
    t_nat_to_T(attn_part, apT, DC, B, "ap")
    ps_xa = psB.tile([P, 512], F32, tag="psB", name="ps_xa")
    for c in range(DC):
        j = c % 4
        nc.tensor.matmul(ps_xa[32 * j:32 * j + B, :HID], apT[:, c, :],
                         rp_sb[:, c, :], start=(c < 4), stop=(c >= DC - 4),
                         tile_position=(0, 32 * j))
    xa_nat = singles.tile([B, HID], F32)
    quad_sum(xa_nat[:], ps_xa, HID)
    nc.vector.tensor_add(out=x0_early[:], in0=x0_early[:], in1=xa_nat[:])

    # ===== STEP 5: y = LN(attn) (affine folded into w1) ; mm1
    y_nat = natp.tile([B, D], F16, tag="natD", name="y_nat")
    layernorm_nat(attn_nat[:], B, D, y_nat[:], "ln0")
    yT = singles.tile([P, DC, B], F16)
    t_nat_to_T(y_nat, yT, DC, B, "y")

    ps_h1 = psA.tile([P, 1024], F32, tag="psA", name="ps_h1")
    for c in range(DC):
        src = (w1_tiles[c // 2][:, c % 2, :] if c < 16
               else w1g_tiles[c - 16][:, :])
        for j in range(4):
            nc.tensor.matmul(
                ps_h1[32 * j:32 * j + B, 0:512],
                yT[:, c, :], src[:, 512 * j:512 * (j + 1)],
                start=(c == 0), stop=False,
                tile_position=(0, 32 * j))
    for j in range(4):
        bias_rep(ps_h1[32 * j:32 * j + B, 0:512], B1_O + 512 * j, 512,
                 (0, 32 * j), stop=True)
    g_nat = natp.tile([B, F1S], F16, tag="nat8", name="g_nat")
    for j in range(4):
        nc.scalar.activation(out=g_nat[:, 512 * j:512 * (j + 1)],
                             in_=ps_h1[32 * j:32 * j + B, 0:512],
                             func=AF.Gelu)
    gT = singles.tile([P, FC, B], F16)
    t_nat_to_T(g_nat, gT, FC, B, "g")

    # ===== STEP 6: x0 += g1 @ w2rin ; AllReduce(x0)
    w2r_sb = w1p.tile([P, FC, HID], F16, tag="w2r", bufs=1, name="w2r")
    nc.sync.dma_start(out=w2r_sb[:], in_=t["w2rin16"][:])
    ps_x0 = psB.tile([P, 512], F32, tag="psB", name="ps_x0")
    for c in range(FC):
        j = c % 4
        nc.tensor.matmul(ps_x0[32 * j:32 * j + B, :HID], gT[:, c, :],
                         w2r_sb[:, c, :], start=(c < 4), stop=(c >= FC - 4),
                         tile_position=(0, 32 * j))
    xg_nat = singles.tile([B, HID], F32)
    quad_sum(xg_nat[:], ps_x0, HID)
    nc.vector.tensor_add(out=x0_early[:], in0=x0_early[:], in1=xg_nat[:])
    nc.gpsimd.dma_start(out=t["cc_x0_in"][:], in_=x0_early[:])
    nc.gpsimd.collective_compute(
        "AllReduce", ALU.add, replica_groups=GROUPS,
        ins=[t["cc_x0_in"][:].opt()], outs=[t["cc_x0_out"][:].opt()])

    # ===== STEP 7: diffusion tail (replicated; blk LN affine folded)
    bw1a = w1p.tile([P, 3, 4 * HID], F16, tag="w1", name="bw1a")
    nc.sync.dma_start(out=bw1a[:], in_=t["bw1a"][:])
    bw1b = w1p.tile([P, 3, 4 * HID], F16, tag="w1", name="bw1b")
    nc.sync.dma_start(out=bw1b[:], in_=t["bw1b"][:])
    bw2a = w1p.tile([P, 12, HID], F16, tag="w1", name="bw2a")
    nc.sync.dma_start(out=bw2a[:], in_=t["bw2a"][:])
    bw2b = w1p.tile([P, 12, HID], F16, tag="w1", name="bw2b")
    nc.sync.dma_start(out=bw2b[:], in_=t["bw2b"][:])

    x_nat = singles.tile([B, HID], F32)
    nc.gpsimd.dma_start(out=x_nat[:], in_=t["cc_x0_out"][:])

    for i in range(NBLK):
        xn = singles.tile([B, HID], F16, name=f"xn_{i}")
        layernorm_nat(x_nat[:], B, HID, xn[:], f"lnb{i}")
        xnT = singles.tile([P, HC, B], F16, name=f"xnT_{i}")
        t_nat_to_T(xn, xnT, HC, B, f"xn{i}")
        ps_bh = psB.tile([P, 512], F32, tag="psB", name=f"ps_bh_{i}")
        for j in range(4):
            for c in range(HC):
                f = 2 * i + c
                src = bw1a if f < 3 else bw1b
                nc.tensor.matmul(
                    ps_bh[32 * j:32 * j + B, 0:256],
                    xnT[:, c, :], src[:, f % 3, 256 * j:256 * (j + 1)],
                    start=(c == 0), stop=False,
                    tile_position=(0, 32 * j))
            bias_rep(ps_bh[32 * j:32 * j + B, 0:256],
                     BB1_O + 1024 * i + 256 * j, 256, (0, 32 * j), stop=True)
        hb = natp.tile([B, 4 * HID], F16, tag="nat8", name=f"hb_{i}")
        for j in range(4):
            nc.scalar.activation(out=hb[:, 256 * j:256 * (j + 1)],
                                 in_=ps_bh[32 * j:32 * j + B, 0:256],
                                 func=AF.Silu)
        hbT = singles.tile([P, 4 * HID // P, B], F16, name=f"hbT_{i}")
        t_nat_to_T(hb, hbT, 4 * HID // P, B, f"hb{i}")

        ps_bo = psB.tile([P, 512], F32, tag="psB", name=f"ps_bo_{i}")
        for c in range(4 * HID // P):
            j = c % 4
            f = 8 * i + c
            src = bw2a if f < 12 else bw2b
            nc.tensor.matmul(ps_bo[32 * j:32 * j + B, :HID], hbT[:, c, :],
                             src[:, f % 12, :],
                             start=(c < 4), stop=(c >= 4 and j != 0),
                             tile_position=(0, 32 * j))
        bias_rep(ps_bo[0:B, :HID], BB2_O + 256 * i, HID, (0, 0), stop=True)
        for q in range(4):
            nc.vector.tensor_add(out=x_nat[:], in0=x_nat[:],
                                 in1=ps_bo[32 * q:32 * q + B, :HID])

    xs = singles.tile([B, HID], F16)
    nc.scalar.activation(out=xs[:], in_=x_nat[:], func=AF.Silu)
    xsT = singles.tile([P, HC, B], F16)
    t_nat_to_T(xs, xsT, HC, B, "xs")
    ps_o = psB.tile([P, 512], F32, tag="psB", name="ps_o")
    for c in range(HC):
        nc.tensor.matmul(ps_o[:B, :AD], xsT[:, c, :], ow_sb[:, c, :],
                         start=(c == 0), stop=(c == HC - 1))
    out_sb = singles.tile([B, AD], F32)
    nc.vector.tensor_add(out=out_sb[:], in0=ps_o[:B, :AD], in1=ob_bc[:])
    nc.sync.dma_start(out=t["res"][:], in_=out_sb[:])


_CACHED_NC = None


def _get_nc():
    global _CACHED_NC
    if _CACHED_NC is None:
        _CACHED_NC = build_program()
    return _CACHED_NC


def _prep_in_maps(inputs):
    f32 = np.float32
    f16 = np.float16
    llm_full = np.asarray(inputs["llm_output"], dtype=f32)
    wq = np.asarray(inputs["wq"], f32); wk = np.asarray(inputs["wk"], f32)
    wv = np.asarray(inputs["wv"], f32); wo = np.asarray(inputs["wo"], f32)
    bq = np.asarray(inputs["bq"], f32); bv = np.asarray(inputs["bv"], f32)
    bo = np.asarray(inputs["bo"], f32)
    ln_g = np.asarray(inputs["ln_g"], f32)
    ln_b = np.asarray(inputs["ln_b"], f32)
    w1 = np.asarray(inputs["mlp_w1"], f32); b1 = np.asarray(inputs["mlp_b1"], f32)
    w2 = np.asarray(inputs["mlp_w2"], f32); b2 = np.asarray(inputs["mlp_b2"], f32)
    rin_w = np.asarray(inputs["rin_w"], f32)
    rin_b = np.asarray(inputs["rin_b"], f32)
    probe = np.asarray(inputs["probe"], f32).reshape(D)
    cw2 = np.asarray(inputs["cond_w2"], f32)
    cb2 = np.asarray(inputs["cond_b2"], f32)
    blk_g = np.asarray(inputs["blk_ln_g"], f32)
    blk_b = np.asarray(inputs["blk_ln_b"], f32)
    blk_w1 = np.asarray(inputs["blk_w1"], f32)
    blk_w2 = np.asarray(inputs["blk_w2"], f32)
    blk_b1 = np.asarray(inputs["blk_b1"], f32)
    blk_b2 = np.asarray(inputs["blk_b2"], f32)

    # ---- weight-only folds ----
    q = (probe @ wq + bq) * RSQRT_DH
    U = np.zeros((D, H), f32)
    for h in range(H):
        U[:, h] = wk[:, h * DH:(h + 1) * DH] @ q[h * DH:(h + 1) * DH]
    U8 = (U * SU).astype(NP8)
    rin_cond = rin_w[0:TD]
    rin_pool = np.ascontiguousarray(rin_w[TD:TD + D])
    rin_na = rin_w[TD + D:]
    w2rin = w2 @ rin_pool
    cw2rin = cw2 @ rin_cond
    rb_fold = (rin_b + b2 @ rin_pool + cb2 @ rin_cond) / NC
    # LN affine folds: y_aff @ W = y_core @ (g*W) + b@W
    w1_aff = ln_g[:, None] * w1              # (D, 4D)
    b1_aff = b1 + ln_b @ w1                  # (4D,)
    bw1_aff = blk_g[:, :, None] * blk_w1     # (3, HID, 4HID)
    bb1_aff = blk_b1 + np.einsum('ih,ihf->if', blk_b, blk_w1)

    def ptile(m, c_per_g):
        K, N = m.shape
        G = K // (P * c_per_g)
        r = np.ascontiguousarray(
            m.reshape(G, c_per_g, P, N).transpose(0, 2, 1, 3))
        return r if G > 1 else r[0]

    shared = {
        "rp16": np.ascontiguousarray(
            rin_pool.reshape(DC, P, HID).transpose(1, 0, 2)).astype(f16),
        "four_w2": np.concatenate(
            [np.asarray(inputs["four_w"], f32).reshape(TD // 2, 1)] * 2),
        "phase2": np.concatenate(
            [np.full((TD // 2, 1), np.pi / 2, f32),
             np.zeros((TD // 2, 1), f32)]),
        "timeT": np.ascontiguousarray(np.asarray(inputs["time"], f32).T),
        "naT": np.ascontiguousarray(
            np.asarray(inputs["noisy_actions"], f32).T).astype(f16),
        "cw1": np.asarray(inputs["cond_w1"], f32).astype(f16),
        "cb1c": np.asarray(inputs["cond_b1"], f32).reshape(-1, 1),
        "cw2rin8": (cw2rin / NC).astype(f16),
        "rna8": (rin_na / NC).astype(f16),
        "rb8": rb_fold.astype(f16).reshape(1, HID),
        "bw1a": np.ascontiguousarray(
            bw1_aff.reshape(NBLK * HC, P, 4 * HID)[0:3].transpose(1, 0, 2)
        ).astype(f16),
        "bw1b": np.ascontiguousarray(
            bw1_aff.reshape(NBLK * HC, P, 4 * HID)[3:6].transpose(1, 0, 2)
        ).astype(f16),
        "bw2a": np.ascontiguousarray(
            blk_w2.reshape(NBLK * 8, P, HID)[0:12].transpose(1, 0, 2)
        ).astype(f16),
        "bw2b": np.ascontiguousarray(
            blk_w2.reshape(NBLK * 8, P, HID)[12:24].transpose(1, 0, 2)
        ).astype(f16),
        "ow": np.ascontiguousarray(
            np.asarray(inputs["out_w"], f32).reshape(HC, P, AD)
            .transpose(1, 0, 2)).astype(f16),
        "out_bc": np.asarray(inputs["out_b"], f32).reshape(1, AD),
        "U8r": np.ascontiguousarray(U8.reshape(DC, P, H).transpose(1, 0, 2)),
    }

    in_maps = []
    for i in range(NC):
        hb_ = slice(i * DH, (i + 1) * DH)
        fb = slice(i * F1S, (i + 1) * F1S)
        m = dict(shared)
        m["llm16"] = llm_full[i].astype(f16)
        m["llmT8"] = np.ascontiguousarray(llm_full[i].T).astype(NP8)
        m["wv16"] = ptile(np.ascontiguousarray(wv[:, hb_]), 4).astype(f16)
        m["wo16"] = np.ascontiguousarray(wo[hb_, :]).astype(f16)
        m["w116"] = ptile(np.ascontiguousarray(w1_aff[:, fb]), 2).astype(f16)
        m["w2rin16"] = ptile(np.ascontiguousarray(w2rin[fb]), FC).astype(f16)
        brep = np.zeros((1, BREP_N), f16)
        brep[0, BO_O:BO_O + D] = (bo / NC).astype(f16)
        brep[0, B1_O:B1_O + F1S] = b1_aff[fb].astype(f16)
        brep[0, BV_O:BV_O + DH] = bv[hb_].astype(f16)
        brep[0, BB1_O:BB1_O + NBLK * 4 * HID] = bb1_aff.reshape(-1).astype(f16)
        brep[0, BB2_O:BB2_O + NBLK * HID] = blk_b2.reshape(-1).astype(f16)
        m["brep"] = brep
        in_maps.append(m)
    return in_maps


def kernel(**inputs):
    nc = _get_nc()
    in_maps = _prep_in_maps(inputs)
    r = run_bass_kernel_spmd(nc, in_maps, core_ids=list(range(NC)))
    return np.ascontiguousarray(r.results[0]["res"]).astype(np.float32)


def run_traced(**inputs):
    nc = _get_nc()
    in_maps = _prep_in_maps(inputs)
    r = run_bass_kernel_spmd(nc, in_maps, core_ids=list(range(NC)), trace=True)
    return np.ascontiguousarray(r.results[0]["res"]).astype(np.float32), r
